# revision 1
# baseline (speedup 1.0000x reference)
"""MultiHeadAttention Trainium2 Bass kernel, 8-core SPMD.

Problem: B=4, S=2048, EMBED=1024, HEADS=16, HEAD_DIM=64 (fp32).
Sharding: core c -> batch b=c//2, query-half h=c%2 (1024 query rows).
Each core computes its 1024 output rows end-to-end; no collectives.

Per-core dataflow (all layouts transposed: feature/kk dim on partitions):
  A1: KT = WkT.T @ XkT + bk              -> SBUF resident  (f32r)
  A2: V  = XvT.T @ WvT (natural [kk,d])  -> SBUF resident bf16, with a
      ones column per head (V_aug) so the PV matmul also produces the
      softmax denominator for free.
  B:  per head-pair p:
      Q-proj for the pair (QT tile stays in SBUF; scale 1/8 + bias
      folded into the PSUM evacuation), then per q-chunk qc(512) and
      kk-pair kkp:
        S.T[kk,q] = KT_h.T @ QT_h        (f32r, heads row-packed in PE)
        P = exp(S.T)                      (ACT, no max-subtraction --
                                           scores ~ N(0,1), safe)
        P *= (1-mask).T                   (DVE, bf16 2x mode)
        OT_h[65,512] += V_aug_h.T @ P     (bf16; row 64 = denominator)
      normalize: OT_h[0:64] * recip(OT_h[64]) -> ot_dram (f32r)
  C:  outT = WoT.T @ OT + (bo + Wo @ bv)  -> DRAM [1024,1024]
Host reassembles out[b, h*1024:(h+1)*1024, :] = outT.T per core.
"""
import numpy as np
import ml_dtypes

import concourse.bass as bass
import concourse.mybir as mybir
import concourse.tile as tile
from concourse import bacc
from concourse.bass_utils import run_bass_kernel_spmd

F32R = mybir.dt.float32r
F32 = mybir.dt.float32
BF16 = mybir.dt.bfloat16
Act = mybir.ActivationFunctionType
Alu = mybir.AluOpType

EMBED = 1024
HEADS = 16
DH = 64
SQ = 1024   # query rows per core
SK = 2048   # key rows per core
NF = 8      # feature tiles (1024/128)
NKT = 16    # kk tiles (2048/128)
N_CORES = 8

_STATE = {}


def build_nc():
    nc = bacc.Bacc("TRN2", target_bir_lowering=False)
    xqT = nc.dram_tensor("xqT", [EMBED, SQ], F32R, kind="ExternalInput")
    xkT = nc.dram_tensor("xkT", [EMBED, SK], F32R, kind="ExternalInput")
    xvT = nc.dram_tensor("xvT", [EMBED, SK], F32R, kind="ExternalInput")
    wqT = nc.dram_tensor("wqT", [EMBED, EMBED], F32R, kind="ExternalInput")
    wkT = nc.dram_tensor("wkT", [EMBED, EMBED], F32R, kind="ExternalInput")
    wvT = nc.dram_tensor("wvT", [EMBED, EMBED], F32R, kind="ExternalInput")
    woT = nc.dram_tensor("woT", [EMBED, EMBED], F32R, kind="ExternalInput")
    bq8 = nc.dram_tensor("bq8", [128, NF], F32, kind="ExternalInput")
    bk_l = nc.dram_tensor("bk_l", [128, NF], F32, kind="ExternalInput")
    bo2 = nc.dram_tensor("bo2", [128, NF], F32, kind="ExternalInput")
    notmT = nc.dram_tensor("notmT", [SK, SQ], BF16, kind="ExternalInput")
    outT = nc.dram_tensor("outT", [EMBED, SQ], F32, kind="ExternalOutput")
    ot_dram = nc.dram_tensor("ot_dram", [EMBED, SQ], F32R)

    xqT_r = xqT.rearrange("(t p) q -> p t q", p=128)
    xkT_r = xkT.rearrange("(t p) k -> p t k", p=128)
    xvT_r = xvT.rearrange("(t p) k -> p t k", p=128)
    wqT_r = wqT.rearrange("(t p) n -> p t n", p=128)
    wkT_r = wkT.rearrange("(t p) n -> p t n", p=128)
    wvT_r = wvT.rearrange("(t p) n -> p t n", p=128)
    woT_r = woT.rearrange("(t p) n -> p t n", p=128)
    notmT_r = notmT.rearrange("(t p) q -> p t q", p=128)
    ot_dram_r = ot_dram.rearrange("(t p) q -> p t q", p=128)

    with tile.TileContext(nc) as tc:
        # ---------- persistent + early-prefetch pools ----------
        with tc.tile_pool(name="persist", bufs=1) as pp, \
             tc.tile_pool(name="bias", bufs=1) as bp, \
             tc.tile_pool(name="xv", bufs=2) as xvpool, \
             tc.tile_pool(name="bwq", bufs=2) as wqpool:
            kt = pp.tile([128, NF, SK], F32R, name="kt")
            vaug = pp.tile([128, NKT, HEADS * 65], BF16, name="vaug")
            bq8_sb = bp.tile([128, NF], F32, name="bq8_sb")
            bk_sb = bp.tile([128, NF], F32, name="bk_sb")
            bo2_sb = bp.tile([128, NF], F32, name="bo2_sb")

            # ---------- phase A1: K projection -> kt ----------
            with tc.tile_pool(name="a1w", bufs=1) as wpool, \
                 tc.tile_pool(name="a1x", bufs=2) as xpool, \
                 tc.tile_pool(name="a1p", bufs=4, space="PSUM") as pspool:
                wk_sb = wpool.tile([128, NF, EMBED], F32R, name="wk_sb")
                for ck in range(4):
                    xk_sb = xpool.tile([128, NF, 512], F32R, name="xk_sb")
                    if ck == 0:
                        nc.sync.dma_start(out=xk_sb[:, 0:2, :],
                                          in_=xkT_r[:, 0:2, 0:512])
                        nc.sync.dma_start(out=wk_sb[:, :, 0:128],
                                          in_=wkT_r[:, :, 0:128])
                        nc.sync.dma_start(out=xk_sb[:, 2:4, :],
                                          in_=xkT_r[:, 2:4, 0:512])
                        nc.sync.dma_start(out=xk_sb[:, 4:8, :],
                                          in_=xkT_r[:, 4:8, 0:512])
                        nc.sync.dma_start(out=wk_sb[:, :, 128:256],
                                          in_=wkT_r[:, :, 128:256])
                        nc.sync.dma_start(out=bk_sb[:], in_=bk_l[:, :])
                        nc.sync.dma_start(out=bq8_sb[:], in_=bq8[:, :])
                        nc.sync.dma_start(out=bo2_sb[:], in_=bo2[:, :])
                        for c4 in range(1, 4):
                            nc.sync.dma_start(
                                out=wk_sb[:, :, c4 * 256:(c4 + 1) * 256],
                                in_=wkT_r[:, :, c4 * 256:(c4 + 1) * 256])
                    else:
                        nc.sync.dma_start(
                            out=xk_sb[:],
                            in_=xkT_r[:, :, ck * 512:(ck + 1) * 512])
                    for m in range(NF):
                        ps = pspool.tile([128, 512], F32, name="a1ps")
                        for fi in range(NF):
                            nc.tensor.matmul(
                                ps[:], wk_sb[:, fi, m * 128:(m + 1) * 128],
                                xk_sb[:, fi, :],
                                start=(fi == 0), stop=(fi == NF - 1))
                        nc.vector.tensor_scalar(
                            out=kt[:, m, ck * 512:(ck + 1) * 512],
                            in0=ps[:], scalar1=bk_sb[:, m:m + 1],
                            scalar2=None, op0=Alu.add)

            # ---------- phase A2: V projection -> vaug (bf16 + ones) ----------
            # n-outer: heads 0..7 (n=0) complete first so phase B's first
            # head-pairs can overlap with the n=1 half.
            vaug_r = vaug.rearrange("p k (h c) -> p k h c", c=65)
            nc.vector.memset(vaug_r[:, :, :, 64:65], 1.0)
            xq_ctx = tc.tile_pool(name="xq", bufs=1)
            xqpool = xq_ctx.__enter__()
            xq_sb = xqpool.tile([128, NF, SQ], F32R, name="xq_sb")
            nc.sync.dma_start(out=xq_sb[:, :, 0:512], in_=xqT_r[:, :, 0:512])
            nc.sync.dma_start(out=xq_sb[:, :, 512:1024],
                              in_=xqT_r[:, :, 512:1024])
            with tc.tile_pool(name="a2w", bufs=2) as wpool, \
                 tc.tile_pool(name="a2p", bufs=4, space="PSUM") as pspool:
                wv_sb = []
                for n in range(2):
                    t = wpool.tile([128, NF, 512], F32R, name="wv_sb")
                    nc.sync.dma_start(out=t[:],
                                      in_=wvT_r[:, :, n * 512:(n + 1) * 512])
                    wv_sb.append(t)
                for m in range(NKT):
                    xv_sb = xvpool.tile([128, NF, 128], F32R,
                                        name="xv_sb")
                    nc.sync.dma_start(
                        out=xv_sb[:],
                        in_=xvT_r[:, :, m * 128:(m + 1) * 128])
                    for n in range(2):
                        ps = pspool.tile([128, 512], F32, name="a2ps")
                        for fi in range(NF):
                            nc.tensor.matmul(
                                ps[:], xv_sb[:, fi, :],
                                wv_sb[n][:, fi, :],
                                start=(fi == 0), stop=(fi == NF - 1))
                        nc.vector.tensor_copy(
                            out=vaug_r[:, m, n * 8:(n + 1) * 8, 0:64],
                            in_=ps.rearrange("p (h c) -> p h c", c=64))

            # ---------- phase B: Q-proj + attention, per head pair ----------
            with tc.tile_pool(name="bnotm", bufs=1) as nmpool, \
                 tc.tile_pool(name="bqt", bufs=2) as qpool, \
                 tc.tile_pool(name="bpt", bufs=2) as ptpool, \
                 tc.tile_pool(name="bnrm", bufs=2) as npool, \
                 tc.tile_pool(name="bst", bufs=1, space="PSUM") as stpool, \
                 tc.tile_pool(name="bqp", bufs=2, space="PSUM") as qppool, \
                 tc.tile_pool(name="bot", bufs=1, space="PSUM") as otpool:
                wq_first = wqpool.tile([128, NF, 128], F32R, name="wq_sb",
                                       tag="wq_sb")
                nc.sync.dma_start(out=wq_first[:], in_=wqT_r[:, :, 0:128])
                notm = nmpool.tile([128, NKT, SQ], BF16, name="notm")
                for c4 in range(4):
                    nc.sync.dma_start(
                        out=notm[:, c4 * 4:(c4 + 1) * 4, :],
                        in_=notmT_r[:, c4 * 4:(c4 + 1) * 4, :])
                for p in range(8):  # head pairs
                    # Q projection for this pair -> qt_sb [128, 1024] f32r
                    if p == 0:
                        wq_sb = wq_first
                    else:
                        wq_sb = wqpool.tile([128, NF, 128], F32R,
                                            name="wq_sb", tag="wq_sb")
                        nc.sync.dma_start(
                            out=wq_sb[:],
                            in_=wqT_r[:, :, p * 128:(p + 1) * 128])
                    qt_sb = qpool.tile([128, SQ], F32R, name="qt_sb")
                    for qc in range(2):
                        qps = qppool.tile([128, 512], F32, name="qps")
                        for fi in range(NF):
                            nc.tensor.matmul(
                                qps[:], wq_sb[:, fi, :],
                                xq_sb[:, fi, qc * 512:(qc + 1) * 512],
                                start=(fi == 0), stop=(fi == NF - 1))
                        nc.vector.tensor_scalar(
                            out=qt_sb[:, qc * 512:(qc + 1) * 512], in0=qps[:],
                            scalar1=0.125, scalar2=bq8_sb[:, p:p + 1],
                            op0=Alu.mult, op1=Alu.add)
                    for qc in range(2):
                        otps = [otpool.tile([128, 512], F32,
                                            name=f"otps{j}", tag=f"otps{j}")
                                for j in range(2)]
                        for kkp in range(8):
                            sts = [stpool.tile([128, 1024], F32,
                                               name=f"stps{j}", tag=f"stps{j}")
                                   for j in range(2)]
                            # ST matmuls interleaved by head so adjacent
                            # PE ops target disjoint row groups (0,0)/(64,0)
                            # and run concurrently (MMs are strict FIFO --
                            # same-row-group neighbors serialize).
                            def st_mm(hh, j):
                                lo = hh * 64
                                kkt = 2 * kkp + j
                                nc.tensor.matmul(
                                    sts[hh][:, j * 512:(j + 1) * 512],
                                    kt[lo:lo + 64, p,
                                       kkt * 128:(kkt + 1) * 128],
                                    qt_sb[lo:lo + 64,
                                          qc * 512:(qc + 1) * 512],
                                    start=True, stop=True,
                                    tile_position=(lo, 0))
                            pts = []
                            st_mm(0, 0)
                            st_mm(1, 0)
                            st_mm(0, 1)
                            pt0 = ptpool.tile([128, 1024], BF16,
                                              name="pt0", tag="pt0")
                            nc.scalar.activation(pt0[:], sts[0][:], Act.Exp)
                            pts.append(pt0)
                            st_mm(1, 1)
                            pt1 = ptpool.tile([128, 1024], BF16,
                                              name="pt1", tag="pt1")
                            nc.scalar.activation(pt1[:], sts[1][:], Act.Exp)
                            pts.append(pt1)
                            for hh in range(2):
                                h = 2 * p + hh
                                for j in range(2):
                                    kkt = 2 * kkp + j
                                    nc.vector.tensor_tensor(
                                        out=pts[hh][:, j * 512:(j + 1) * 512],
                                        in0=pts[hh][:, j * 512:(j + 1) * 512],
                                        in1=notm[:, kkt,
                                                 qc * 512:(qc + 1) * 512],
                                        op=Alu.mult)
                                    nc.tensor.matmul(
                                        otps[hh][0:65, :],
                                        vaug_r[:, kkt, h, :],
                                        pts[hh][:, j * 512:(j + 1) * 512],
                                        start=(kkp == 0 and j == 0),
                                        stop=(kkp == 7 and j == 1))
                        for hh in range(2):
                            rec = npool.tile([1, 512], F32, name="rec",
                                             tag="rec")
                            nc.vector.reciprocal(rec[:], otps[hh][64:65, :])
                            recb = npool.tile([64, 512], F32, name="recb",
                                              tag="recb")
                            nc.gpsimd.partition_broadcast(recb[:], rec[:])
                            otstg = npool.tile([64, 512], F32R, name="otstg",
                                               tag="otstg")
                            nc.vector.tensor_tensor(
                                out=otstg[:],
                                in0=otps[hh][0:64, :], in1=recb[:],
                                op=Alu.mult)
                            nc.sync.dma_start(
                                out=ot_dram[p * 128 + hh * 64:
                                            p * 128 + hh * 64 + 64,
                                            qc * 512:(qc + 1) * 512],
                                in_=otstg[:])

            # ---------- phase C: output projection ----------
            # wo streams through the (still open) bwq pool so the first
            # blocks prefetch during phase B's tail.
            with tc.tile_pool(name="cot", bufs=1) as cotpool, \
                 tc.tile_pool(name="cs", bufs=3) as spool, \
                 tc.tile_pool(name="cp", bufs=4, space="PSUM") as pspool:
                ot_sb = []
                for qc in range(2):
                    t = cotpool.tile([128, NF, 512], F32R, name=f"ot_sb{qc}")
                    if qc == 0:
                        nc.sync.dma_start(out=t[:, 0:4, :],
                                          in_=ot_dram_r[:, 0:4, 0:512])
                        nc.sync.dma_start(out=t[:, 4:8, :],
                                          in_=ot_dram_r[:, 4:8, 0:512])
                    else:
                        nc.sync.dma_start(
                            out=t[:],
                            in_=ot_dram_r[:, :, qc * 512:(qc + 1) * 512])
                    ot_sb.append(t)
                for m in range(NF):
                    wo_sb = wqpool.tile([128, NF, 128], F32R, name="wo_sb",
                                        tag="wq_sb")
                    nc.sync.dma_start(
                        out=wo_sb[:],
                        in_=woT_r[:, :, m * 128:(m + 1) * 128])
                    for qc in range(2):
                        ps = pspool.tile([128, 512], F32, name="cps")
                        for fi in range(NF):
                            nc.tensor.matmul(
                                ps[:], wo_sb[:, fi, :],
                                ot_sb[qc][:, fi, :],
                                start=(fi == 0), stop=(fi == NF - 1))
                        stg = spool.tile([128, 512], F32, name="cstg")
                        nc.vector.tensor_scalar(
                            out=stg[:], in0=ps[:],
                            scalar1=bo2_sb[:, m:m + 1], scalar2=None,
                            op0=Alu.add)
                        nc.sync.dma_start(
                            out=outT[m * 128:(m + 1) * 128,
                                     qc * 512:(qc + 1) * 512],
                            in_=stg[:])
            xq_ctx.__exit__(None, None, None)
    nc.compile()
    return nc


def _get_nc():
    if "nc" not in _STATE:
        _STATE["nc"] = build_nc()
    return _STATE["nc"]


def kernel(query, key, value, mask, Wq, bq, Wk, bk, Wv, bv, Wo, bo):
    query = np.asarray(query, dtype=np.float32)
    key = np.asarray(key, dtype=np.float32)
    value = np.asarray(value, dtype=np.float32)
    mask = np.asarray(mask)
    Wq = np.asarray(Wq, dtype=np.float32)
    Wk = np.asarray(Wk, dtype=np.float32)
    Wv = np.asarray(Wv, dtype=np.float32)
    Wo = np.asarray(Wo, dtype=np.float32)
    bq = np.asarray(bq, dtype=np.float32)
    bk = np.asarray(bk, dtype=np.float32)
    bv = np.asarray(bv, dtype=np.float32)
    bo = np.asarray(bo, dtype=np.float32)

    wqT = np.ascontiguousarray(Wq.T)
    wkT = np.ascontiguousarray(Wk.T)
    wvT = np.ascontiguousarray(Wv.T)
    woT = np.ascontiguousarray(Wo.T)
    bq8 = np.ascontiguousarray((bq / 8.0).reshape(NF, 128).T)
    bk_l = np.ascontiguousarray(bk.reshape(NF, 128).T)
    bo2v = bo + Wo @ bv
    bo2 = np.ascontiguousarray(bo2v.reshape(NF, 128).T)

    in_maps = []
    for c in range(N_CORES):
        b, h = c // 2, c % 2
        rows = slice(h * SQ, (h + 1) * SQ)
        xqTc = np.ascontiguousarray(query[b, rows, :].T)
        xkTc = np.ascontiguousarray(key[b].T)
        xvTc = np.ascontiguousarray(value[b].T)
        notm = np.ascontiguousarray(
            (~mask[b, 0, rows, :]).T.astype(ml_dtypes.bfloat16))
        in_maps.append({
            "xqT": xqTc, "xkT": xkTc, "xvT": xvTc,
            "wqT": wqT, "wkT": wkT, "wvT": wvT, "woT": woT,
            "bq8": bq8, "bk_l": bk_l, "bo2": bo2,
            "notmT": notm,
        })

    nc = _get_nc()
    res = run_bass_kernel_spmd(nc, in_maps, core_ids=list(range(N_CORES)))
    out = np.empty((4, 2048, EMBED), dtype=np.float32)
    for c in range(N_CORES):
        b, h = c // 2, c % 2
        out[b, h * SQ:(h + 1) * SQ, :] = res.results[c]["outT"].T
    return out



# revision 4
# speedup vs baseline: 1.2041x; 1.2041x over previous
"""MultiHeadAttention TRN2 kernel, 8-core SPMD — interleaved emission (V3.1).

Sharding: core c -> batch b=c//2, head-group hg=c%2 (8 heads / 512 dims).
All inputs bf16 except biases. Spine is q-tile-major attention; K/Q/V
projections and the q-tile-0 output projection are "extras" interleaved
into the PE stream so the ACT engine (exp, the ~266us floor) stays fed.
PV matmuls trail the ST/exp stream through a pending queue (P stationary,
free=65 -> half PE cost) and may slip across segment boundaries.

PSUM accumulation groups are per 2KB zero region (bank): start zeroes the
whole bank -> one start/stop per bank, interior matmuls skip group check.
Extras borrow PSUM from the st pool (ov pool slots live too long).
"""
import numpy as np
import ml_dtypes

import concourse.bass as bass
import concourse.mybir as mybir
import concourse.tile as tile
from concourse import bacc
from concourse.bass_utils import run_bass_kernel_spmd

F32 = mybir.dt.float32
BF16 = mybir.dt.bfloat16
Act = mybir.ActivationFunctionType
Alu = mybir.AluOpType

EMBED = 1024
SK = 2048
SQ = 2048
NF = 8
NKT = 16
NP = 4
N_CORES = 8
LAG = 4

_STATE = {}


def build_nc():
    nc = bacc.Bacc("TRN2", target_bir_lowering=False)
    xqT = nc.dram_tensor("xqT", [EMBED, SQ], BF16, kind="ExternalInput")
    xkT = nc.dram_tensor("xkT", [EMBED, SK], BF16, kind="ExternalInput")
    xvT = nc.dram_tensor("xvT", [EMBED, SK], BF16, kind="ExternalInput")
    wqT = nc.dram_tensor("wqT", [EMBED, 512], BF16, kind="ExternalInput")
    wkT = nc.dram_tensor("wkT", [EMBED, 512], BF16, kind="ExternalInput")
    wvT = nc.dram_tensor("wvT", [EMBED, 512], BF16, kind="ExternalInput")
    woT = nc.dram_tensor("woT", [512, EMBED], BF16, kind="ExternalInput")
    bq8 = nc.dram_tensor("bq8", [128, NP], F32, kind="ExternalInput")
    bk_l = nc.dram_tensor("bk_l", [128, NP], F32, kind="ExternalInput")
    bo2 = nc.dram_tensor("bo2", [128, NF], F32, kind="ExternalInput")
    notmT = nc.dram_tensor("notmT", [SK, SQ], BF16, kind="ExternalInput")
    outT = nc.dram_tensor("outT", [EMBED, SQ], F32, kind="ExternalOutput")

    xqT_r = xqT.rearrange("(t p) q -> p t q", p=128)
    xkT_r = xkT.rearrange("(t p) k -> p t k", p=128)
    xvT_r = xvT.rearrange("(t p) k -> p t k", p=128)
    wqT_r = wqT.rearrange("(t p) n -> p t n", p=128)
    wkT_r = wkT.rearrange("(t p) n -> p t n", p=128)
    wvT_r = wvT.rearrange("(t p) n -> p t n", p=128)
    woT_r = woT.rearrange("(t p) n -> p t n", p=128)
    notmT_r = notmT.rearrange("(t p) q -> p t q", p=128)

    with tile.TileContext(nc) as tc:
        with tc.tile_pool(name="persist", bufs=1) as pp, \
             tc.tile_pool(name="bias", bufs=1) as bp, \
             tc.tile_pool(name="nmch", bufs=8) as nmpool, \
             tc.tile_pool(name="wkch", bufs=2) as wkpool, \
             tc.tile_pool(name="xqch", bufs=4) as xqpool, \
             tc.tile_pool(name="xvch", bufs=2) as xvpool, \
             tc.tile_pool(name="apt", bufs=7) as ptpool, \
             tc.tile_pool(name="aon", bufs=1) as onpool, \
             tc.tile_pool(name="aotn", bufs=2) as otnpool, \
             tc.tile_pool(name="aost", bufs=3) as ostg, \
             tc.tile_pool(name="ast", bufs=2, space="PSUM") as stpool, \
             tc.tile_pool(name="axps", bufs=2, space="PSUM") as xpspool, \
             tc.tile_pool(name="aov", bufs=1, space="PSUM") as ovpool:
            kt = pp.tile([128, NP, SK], BF16, name="kt")
            qt = pp.tile([128, NP, SQ], BF16, name="qt")
            vaug = pp.tile([128, NKT, 8, 65], BF16, name="vaug")
            xk_sb = pp.tile([128, NF, SK], BF16, name="xk_sb")
            wq_sb = pp.tile([128, NF, 512], BF16, name="wq_sb")
            wv_sb = pp.tile([128, NF, 512], BF16, name="wv_sb")
            wo_sb = pp.tile([128, NP, EMBED], BF16, name="wo_sb")
            ott = pp.tile([128, NP, SQ], BF16, name="ott")
            bq8_sb = bp.tile([128, NP], F32, name="bq8_sb")
            bk_sb = bp.tile([128, NP], F32, name="bk_sb")
            bo2_sb = bp.tile([128, NF], F32, name="bo2_sb")
            nc.vector.memset(vaug[:, :, :, 64:65], 1.0)

            # ---------------- DMA preamble (urgency-ordered) ----------
            nc.sync.dma_start(out=bk_sb[:], in_=bk_l[:, :])
            nc.sync.dma_start(out=bq8_sb[:], in_=bq8[:, :])
            nc.sync.dma_start(out=bo2_sb[:], in_=bo2[:, :])
            wk_t = {}
            wk_t[0] = wkpool.tile([128, NF, 128], BF16, name="wk0", tag="wk")
            for f4 in range(2):
                nc.sync.dma_start(out=wk_t[0][:, 4 * f4:4 * f4 + 4, :],
                                  in_=wkT_r[:, 4 * f4:4 * f4 + 4, 0:128])
            # xk block 0 fine-grained (feeds K(p0,b0) asap)
            for f in range(NF):
                nc.scalar.dma_start(out=xk_sb[:, f, 0:512],
                                    in_=xkT_r[:, f, 0:512])
            # wq cols 0:256 (pairs 0,1) first
            for f4 in range(2):
                nc.sync.dma_start(out=wq_sb[:, 4 * f4:4 * f4 + 4, 0:256],
                                  in_=wqT_r[:, 4 * f4:4 * f4 + 4, 0:256])
            xq_t = {}
            for ck in range(4):
                xq_t[ck] = xqpool.tile([128, NF, 256], BF16,
                                       name=f"xq{ck}", tag="xq")
                for f4 in range(2):
                    nc.sync.dma_start(
                        out=xq_t[ck][:, 4 * f4:4 * f4 + 4, :],
                        in_=xqT_r[:, 4 * f4:4 * f4 + 4,
                                  ck * 256:(ck + 1) * 256])
            # mask chunk 0 + V inputs early (V(0) pulled ~slot 4)
            nm_t = {}
            def _nmload0(kkg):
                t = nmpool.tile([128, 2, 1024], BF16, name=f"nm0{kkg}",
                                tag="nm")
                nm_t[(0, kkg)] = t
                for half in range(2):
                    nc.scalar.dma_start(
                        out=t[:, half:half + 1, :],
                        in_=notmT_r[:, kkg * 2 + half:kkg * 2 + half + 1,
                                    0:1024])
            _nmload0(0)
            xv_t = {}
            for kkt in range(2):
                xv_t[kkt] = xvpool.tile([128, NF, 128], BF16,
                                        name=f"xv{kkt}", tag="xv")
                nc.scalar.dma_start(
                    out=xv_t[kkt][:],
                    in_=xvT_r[:, :, kkt * 128:(kkt + 1) * 128])
            for f2 in range(4):
                nc.sync.dma_start(
                    out=wv_sb[:, 2 * f2:2 * f2 + 2, :],
                    in_=wvT_r[:, 2 * f2:2 * f2 + 2, :])
            _nmload0(1)
            # xk block 1 (K(p0,b1) at slot ~1)
            for f2 in range(4):
                nc.scalar.dma_start(
                    out=xk_sb[:, 2 * f2:2 * f2 + 2, 512:1024],
                    in_=xkT_r[:, 2 * f2:2 * f2 + 2, 512:1024])
            _nmload0(2)
            _nmload0(3)
            # wq cols 256:512 (pairs 2,3 - needed from h4)
            for f4 in range(2):
                nc.sync.dma_start(out=wq_sb[:, 4 * f4:4 * f4 + 4, 256:512],
                                  in_=wqT_r[:, 4 * f4:4 * f4 + 4, 256:512])
            for kkg in range(4, 8):
                _nmload0(kkg)
            # xk blocks 2-3, wo (bulk, later deadlines)
            for blk in range(2, 4):
                for f4 in range(2):
                    nc.scalar.dma_start(
                        out=xk_sb[:, 4 * f4:4 * f4 + 4,
                                  blk * 512:(blk + 1) * 512],
                        in_=xkT_r[:, 4 * f4:4 * f4 + 4,
                                  blk * 512:(blk + 1) * 512])
            for f4 in range(2):
                nc.scalar.dma_start(
                    out=wo_sb[:, 2 * f4:2 * f4 + 2, :],
                    in_=woT_r[:, 2 * f4:2 * f4 + 2, :])

            # ---------------- thunks (each ~1.7us of PE) ----------------
            def exps(shape_view):
                ps = stpool.tile([128, 1024], F32, name="xps")
                return ps, ps[:, 0:shape_view].rearrange("p a -> p a")

            def kproj_thunk(p, blk):
                def run():
                    if blk == 0 and p + 1 < NP:
                        wk_t[p + 1] = wkpool.tile([128, NF, 128], BF16,
                                                  name=f"wk{p+1}", tag="wk")
                        nc.sync.dma_start(
                            out=wk_t[p + 1][:],
                            in_=wkT_r[:, :, (p + 1) * 128:(p + 2) * 128])
                    psv = xpspool.tile([128, 512], F32, name="kps",
                                        tag="xps")
                    for fi in range(NF):
                        nc.tensor.matmul(
                            psv, wk_t[p][:, fi, :],
                            xk_sb[:, fi, blk * 512:(blk + 1) * 512],
                            start=(fi == 0), stop=(fi == NF - 1))
                    nc.vector.tensor_scalar(
                        out=kt[:, p, blk * 512:(blk + 1) * 512],
                        in0=psv, scalar1=bk_sb[:, p:p + 1],
                        scalar2=None, op0=Alu.add)
                return run

            def qproj_thunk(ck, pgrp):
                def run():
                    if pgrp == 1 and ck + 4 < 8:
                        nck = ck + 4
                        xq_t[nck] = xqpool.tile([128, NF, 256], BF16,
                                                name=f"xq{nck}", tag="xq")
                        nc.sync.dma_start(
                            out=xq_t[nck][:],
                            in_=xqT_r[:, :, nck * 256:(nck + 1) * 256])
                    for p in (2 * pgrp, 2 * pgrp + 1):
                        ps = xpspool.tile([128, 512], F32, name="qps",
                                          tag="xps")
                        psv = ps[:, 0:256]
                        for fi in range(NF):
                            nc.tensor.matmul(
                                psv, wq_sb[:, fi, p * 128:(p + 1) * 128],
                                xq_t[ck][:, fi, :],
                                start=(fi == 0), stop=(fi == NF - 1))
                        nc.vector.tensor_scalar(
                            out=qt[:, p, ck * 256:(ck + 1) * 256],
                            in0=psv, scalar1=0.125,
                            scalar2=bq8_sb[:, p:p + 1],
                            op0=Alu.mult, op1=Alu.add)
                return run

            def vproj_thunk(kkt):
                def run():
                    if kkt + 2 < NKT:
                        xv_t[kkt + 2] = xvpool.tile([128, NF, 128], BF16,
                                                    name=f"xv{kkt+2}",
                                                    tag="xv")
                        nc.scalar.dma_start(
                            out=xv_t[kkt + 2][:],
                            in_=xvT_r[:, :, (kkt + 2) * 128:(kkt + 3) * 128])
                    psv = xpspool.tile([128, 512], F32, name="vps",
                                        tag="xps")
                    for fi in range(NF):
                        nc.tensor.matmul(
                            psv, xv_t[kkt][:, fi, :], wv_sb[:, fi, :],
                            start=(fi == 0), stop=(fi == NF - 1))
                    nc.vector.tensor_copy(
                        out=vaug[:, kkt, :, 0:64],
                        in_=psv.rearrange("p (h d) -> p h d", d=64))
                    emitted_v.add(kkt)
                return run

            def nmload_thunk(kkg):
                def run():
                    t = nmpool.tile([128, 2, 1024], BF16, name=f"nm1{kkg}",
                                    tag="nm")
                    nm_t[(1, kkg)] = t
                    for half in range(2):
                        nc.sync.dma_start(
                            out=t[:, half:half + 1, :],
                            in_=notmT_r[:, kkg * 2 + half:
                                        kkg * 2 + half + 1, 1024:2048])
                return run

            def outproj_thunk(qti, et):
                qoff = qti * 1024
                def run():
                    for qh in range(2):
                        psv = xpspool.tile([128, 512], F32, name="ops",
                                            tag="xps")
                        for dc in range(NP):
                            nc.tensor.matmul(
                                psv, wo_sb[:, dc, et * 128:(et + 1) * 128],
                                ott[:, dc,
                                    qoff + qh * 512:qoff + (qh + 1) * 512],
                                start=(dc == 0), stop=(dc == NP - 1))
                        stg = ostg.tile([128, 512], F32, name="cstg")
                        nc.vector.tensor_scalar(
                            out=stg[:], in0=psv,
                            scalar1=bo2_sb[:, et:et + 1], scalar2=None,
                            op0=Alu.add)
                        for qq in range(2):
                            nc.sync.dma_start(
                                out=outT[et * 128:(et + 1) * 128,
                                         qoff + qh * 512 + qq * 256:
                                         qoff + qh * 512 + (qq + 1) * 256],
                                in_=stg[:, qq * 256:(qq + 1) * 256])
                return run

            emitted_v = set()

            def vproj_if_needed(kkt):
                if kkt not in emitted_v:
                    vproj_thunk(kkt)()

            # ---------------- upfront PE: minimum to start h0 ------------
            kproj_thunk(0, 0)()
            for ck in range(4):
                qproj_thunk(ck, 0)()

            # extras: {(qti,h): [(slot, thunk), ...]} (V is pulled on demand
            # by the PV drain, so only K/Q/nm/outproj are slotted here)
            X = {
                (0, 0): [(1, kproj_thunk(0, 1)), (5, kproj_thunk(0, 2)),
                         (9, kproj_thunk(0, 3))],
                (0, 1): [(0, kproj_thunk(1, 0)), (4, kproj_thunk(1, 1)),
                         (8, kproj_thunk(1, 2)), (12, kproj_thunk(1, 3))],
                (0, 2): [(2, kproj_thunk(2, 0)), (6, kproj_thunk(2, 1)),
                         (10, kproj_thunk(2, 2)), (14, kproj_thunk(2, 3))],
                (0, 3): [(1, qproj_thunk(0, 1)), (5, qproj_thunk(1, 1)),
                         (9, qproj_thunk(2, 1)), (13, qproj_thunk(3, 1))],
                (0, 4): [(1, kproj_thunk(3, 0)), (5, kproj_thunk(3, 1)),
                         (9, kproj_thunk(3, 2)), (13, kproj_thunk(3, 3))],
                (0, 5): [(1, qproj_thunk(4, 0)), (5, qproj_thunk(4, 1)),
                         (9, qproj_thunk(5, 0)), (13, qproj_thunk(5, 1)),
                         (11, nmload_thunk(0)), (15, nmload_thunk(1))],
                (0, 6): [(1, qproj_thunk(6, 0)), (5, qproj_thunk(6, 1)),
                         (3, nmload_thunk(2)), (7, nmload_thunk(3)),
                         (9, nmload_thunk(4)), (13, nmload_thunk(5))],
                (0, 7): [(1, qproj_thunk(7, 0)), (5, qproj_thunk(7, 1)),
                         (9, nmload_thunk(6)), (13, nmload_thunk(7))],
                (1, 0): [(2, outproj_thunk(0, 0)), (10, outproj_thunk(0, 1))],
                (1, 1): [(2, outproj_thunk(0, 2)), (10, outproj_thunk(0, 3))],
                (1, 2): [(2, outproj_thunk(0, 4)), (10, outproj_thunk(0, 5))],
                (1, 3): [(2, outproj_thunk(0, 6)), (10, outproj_thunk(0, 7))],
            }

            # ---------------- spine with pending-PV queue ----------------
            pvq = []   # entries: [qti, h, kkt, pt, ov, remaining-first-flag]
            seg_left = {}  # (qti,h) -> count of unemitted PVs

            def head_finish(qti, h, ov, otn):
                p, lo = h // 2, (h % 2) * 64
                qoff = qti * 1024
                onorm = onpool.tile([128, 8, 65], F32, name="onorm")
                nc.vector.tensor_copy(out=onorm[:], in_=ov[:, :, 0:65])
                for qs in range(8):
                    nc.gpsimd.normalize_recip(
                        otn[:, qs, lo:lo + 64],
                        onorm[:, qs, 0:64],
                        onorm[:, qs, 64:65])
                if h % 2 == 1:
                    for qs in range(8):
                        nc.sync.dma_start_transpose(
                            ott[:, p,
                                qoff + qs * 128:qoff + (qs + 1) * 128],
                            otn[:, qs, :])

            def emit_one_pv():
                qti, h, kkt, ptt, ov, otn = pvq.pop(0)
                for qs in range(8):
                    nc.tensor.matmul(
                        ov[:, qs, 0:65],
                        ptt[:, qs * 128:(qs + 1) * 128],
                        vaug[:, kkt, h, :],
                        start=(kkt == 0 and qs % 4 == 0),
                        stop=(kkt == NKT - 1 and qs % 4 == 3),
                        skip_group_check=qs % 4 not in (0, 3))
                seg_left[(qti, h)] -= 1
                if seg_left[(qti, h)] == 0:
                    head_finish(qti, h, ov, otn)

            def drain_pv(limit_pending):
                while len(pvq) > limit_pending:
                    vproj_if_needed(pvq[0][2])
                    emit_one_pv()

            otn = None
            for qti in range(2):
                qoff = qti * 1024
                for h in range(8):
                    p, lo = h // 2, (h % 2) * 64
                    slots = [[] for _ in range(16)]
                    for s, th in X.get((qti, h), []):
                        slots[s].append(th)
                    ov = ovpool.tile([128, 8, 128], F32, name="ov", tag="ov")
                    if h % 2 == 0:
                        otn = otnpool.tile([128, 8, 128], BF16,
                                           name="otn", tag="otn")
                    seg_left[(qti, h)] = NKT
                    lag = 0 if (qti, h) == (1, 7) else LAG
                    for kkt in range(NKT):
                        drain_pv(lag)
                        st = stpool.tile([128, 1024], F32, name="st", tag="st")
                        for j in range(2):
                            nc.tensor.matmul(
                                st[:, j * 512:(j + 1) * 512],
                                kt[lo:lo + 64, p, kkt * 128:(kkt + 1) * 128],
                                qt[lo:lo + 64, p,
                                   qoff + j * 512:qoff + (j + 1) * 512],
                                start=True, stop=True,
                                tile_position=(lo, 0))
                        pt = ptpool.tile([128, 1024], BF16, name="pt")
                        nc.scalar.activation(pt[:], st[:], Act.Exp)
                        nmt = nm_t[(qti, kkt // 2)]
                        nc.vector.tensor_tensor(
                            out=pt[:], in0=pt[:],
                            in1=nmt[:, kkt % 2, :], op=Alu.mult)
                        pvq.append([qti, h, kkt, pt, ov, otn])
                        for th in slots[kkt]:
                            th()
                    drain_pv(0)
            # tail: q-tile-1 output projection
            for et in range(NF):
                outproj_thunk(1, et)()
    nc.compile()
    return nc


def _get_nc():
    if "nc" not in _STATE:
        _STATE["nc"] = build_nc()
    return _STATE["nc"]


def kernel(query, key, value, mask, Wq, bq, Wk, bk, Wv, bv, Wo, bo):
    query = np.asarray(query, dtype=np.float32)
    key = np.asarray(key, dtype=np.float32)
    value = np.asarray(value, dtype=np.float32)
    mask = np.asarray(mask)
    Wq = np.asarray(Wq, dtype=np.float32)
    Wk = np.asarray(Wk, dtype=np.float32)
    Wv = np.asarray(Wv, dtype=np.float32)
    Wo = np.asarray(Wo, dtype=np.float32)
    bq = np.asarray(bq, dtype=np.float32)
    bk = np.asarray(bk, dtype=np.float32)
    bv = np.asarray(bv, dtype=np.float32)
    bo = np.asarray(bo, dtype=np.float32)

    bf = ml_dtypes.bfloat16
    xq_b, xk_b, xv_b, nm_b = [], [], [], []
    for b in range(4):
        xq_b.append(np.ascontiguousarray(query[b].T).astype(bf))
        xk_b.append(np.ascontiguousarray(key[b].T).astype(bf))
        xv_b.append(np.ascontiguousarray(value[b].T).astype(bf))
        nm_b.append(np.ascontiguousarray(
            (~mask[b, 0]).T.astype(bf)))
    wq_g, wk_g, wv_g, wo_g, bq_g, bk_g, bo_g = [], [], [], [], [], [], []
    for hg in range(2):
        dsl = slice(hg * 512, (hg + 1) * 512)
        wq_g.append(np.ascontiguousarray(Wq.T[:, dsl]).astype(bf))
        wk_g.append(np.ascontiguousarray(Wk.T[:, dsl]).astype(bf))
        wv_g.append(np.ascontiguousarray(Wv.T[:, dsl]).astype(bf))
        wo_g.append(np.ascontiguousarray(Wo[:, dsl].T).astype(bf))
        bq_g.append(np.ascontiguousarray((bq[dsl] / 8.0).reshape(NP, 128).T))
        bk_g.append(np.ascontiguousarray(bk[dsl].reshape(NP, 128).T))
        bo_g.append(np.ascontiguousarray(
            (bo / 2.0 + Wo[:, dsl] @ bv[dsl]).reshape(NF, 128).T))

    in_maps = []
    for c in range(N_CORES):
        b, hg = c // 2, c % 2
        in_maps.append({
            "xqT": xq_b[b], "xkT": xk_b[b], "xvT": xv_b[b],
            "wqT": wq_g[hg], "wkT": wk_g[hg], "wvT": wv_g[hg],
            "woT": wo_g[hg],
            "bq8": bq_g[hg], "bk_l": bk_g[hg], "bo2": bo_g[hg],
            "notmT": nm_b[b],
        })

    nc = _get_nc()
    res = run_bass_kernel_spmd(nc, in_maps, core_ids=list(range(N_CORES)))
    out = np.empty((4, SQ, EMBED), dtype=np.float32)
    for b in range(4):
        acc = res.results[2 * b]["outT"] + res.results[2 * b + 1]["outT"]
        out[b] = acc.T
    return out


# revision 5
# speedup vs baseline: 1.2119x; 1.0065x over previous
"""MultiHeadAttention TRN2 kernel, 8-core SPMD — interleaved emission (V3.1).

Sharding: core c -> batch b=c//2, head-group hg=c%2 (8 heads / 512 dims).
All inputs bf16 except biases. Spine is q-tile-major attention; K/Q/V
projections and the q-tile-0 output projection are "extras" interleaved
into the PE stream so the ACT engine (exp, the ~266us floor) stays fed.
PV matmuls trail the ST/exp stream through a pending queue (P stationary,
free=65 -> half PE cost) and may slip across segment boundaries.

PSUM accumulation groups are per 2KB zero region (bank): start zeroes the
whole bank -> one start/stop per bank, interior matmuls skip group check.
Extras borrow PSUM from the st pool (ov pool slots live too long).
"""
import numpy as np
import ml_dtypes

import concourse.bass as bass
import concourse.mybir as mybir
import concourse.tile as tile
from concourse import bacc
from concourse.bass_utils import run_bass_kernel_spmd

F32 = mybir.dt.float32
BF16 = mybir.dt.bfloat16
Act = mybir.ActivationFunctionType
Alu = mybir.AluOpType

EMBED = 1024
SK = 2048
SQ = 2048
NF = 8
NKT = 16
NP = 4
N_CORES = 8
LAG = 4

_STATE = {}


def build_nc():
    nc = bacc.Bacc("TRN2", target_bir_lowering=False)
    xqT = nc.dram_tensor("xqT", [EMBED, SQ], BF16, kind="ExternalInput")
    xkT = nc.dram_tensor("xkT", [EMBED, SK], BF16, kind="ExternalInput")
    xvT = nc.dram_tensor("xvT", [EMBED, SK], BF16, kind="ExternalInput")
    wqT = nc.dram_tensor("wqT", [EMBED, 512], BF16, kind="ExternalInput")
    wkT = nc.dram_tensor("wkT", [EMBED, 512], BF16, kind="ExternalInput")
    wvT = nc.dram_tensor("wvT", [EMBED, 512], BF16, kind="ExternalInput")
    woT = nc.dram_tensor("woT", [512, EMBED], BF16, kind="ExternalInput")
    bq8 = nc.dram_tensor("bq8", [128, NP], F32, kind="ExternalInput")
    bk_l = nc.dram_tensor("bk_l", [128, NP], F32, kind="ExternalInput")
    bo2 = nc.dram_tensor("bo2", [128, NF], F32, kind="ExternalInput")
    notmT = nc.dram_tensor("notmT", [SK, SQ], BF16, kind="ExternalInput")
    outT = nc.dram_tensor("outT", [EMBED, SQ], F32, kind="ExternalOutput")

    xqT_r = xqT.rearrange("(t p) q -> p t q", p=128)
    xkT_r = xkT.rearrange("(t p) k -> p t k", p=128)
    xvT_r = xvT.rearrange("(t p) k -> p t k", p=128)
    wqT_r = wqT.rearrange("(t p) n -> p t n", p=128)
    wkT_r = wkT.rearrange("(t p) n -> p t n", p=128)
    wvT_r = wvT.rearrange("(t p) n -> p t n", p=128)
    woT_r = woT.rearrange("(t p) n -> p t n", p=128)
    notmT_r = notmT.rearrange("(t p) q -> p t q", p=128)

    with tile.TileContext(nc) as tc:
        with tc.tile_pool(name="persist", bufs=1) as pp, \
             tc.tile_pool(name="bias", bufs=1) as bp, \
             tc.tile_pool(name="nmch", bufs=8) as nmpool, \
             tc.tile_pool(name="wkch", bufs=2) as wkpool, \
             tc.tile_pool(name="xqch", bufs=4) as xqpool, \
             tc.tile_pool(name="xvch", bufs=2) as xvpool, \
             tc.tile_pool(name="apt", bufs=7) as ptpool, \
             tc.tile_pool(name="aon", bufs=1) as onpool, \
             tc.tile_pool(name="aotn", bufs=2) as otnpool, \
             tc.tile_pool(name="aost", bufs=3) as ostg, \
             tc.tile_pool(name="ast", bufs=2, space="PSUM") as stpool, \
             tc.tile_pool(name="axps", bufs=2, space="PSUM") as xpspool, \
             tc.tile_pool(name="aov", bufs=1, space="PSUM") as ovpool:
            kt = pp.tile([128, NP, SK], BF16, name="kt")
            qt = pp.tile([128, NP, SQ], BF16, name="qt")
            vaug = pp.tile([128, NKT, 8, 65], BF16, name="vaug")
            xk_sb = pp.tile([128, NF, SK], BF16, name="xk_sb")
            wq_sb = pp.tile([128, NF, 512], BF16, name="wq_sb")
            wv_sb = pp.tile([128, NF, 512], BF16, name="wv_sb")
            wo_sb = pp.tile([128, NP, EMBED], BF16, name="wo_sb")
            ott = pp.tile([128, NP, SQ], BF16, name="ott")
            bq8_sb = bp.tile([128, NP], F32, name="bq8_sb")
            bk_sb = bp.tile([128, NP], F32, name="bk_sb")
            bo2_sb = bp.tile([128, NF], F32, name="bo2_sb")
            nc.vector.memset(vaug[:, :, :, 64:65], 1.0)

            # ---------------- DMA preamble (urgency-ordered) ----------
            nc.sync.dma_start(out=bk_sb[:], in_=bk_l[:, :])
            nc.sync.dma_start(out=bq8_sb[:], in_=bq8[:, :])
            nc.sync.dma_start(out=bo2_sb[:], in_=bo2[:, :])
            wk_t = {}
            wk_t[0] = wkpool.tile([128, NF, 128], BF16, name="wk0", tag="wk")
            for f4 in range(2):
                nc.sync.dma_start(out=wk_t[0][:, 4 * f4:4 * f4 + 4, :],
                                  in_=wkT_r[:, 4 * f4:4 * f4 + 4, 0:128])
            # xk block 0 fine-grained (feeds K(p0,b0) asap)
            for f in range(NF):
                nc.scalar.dma_start(out=xk_sb[:, f, 0:512],
                                    in_=xkT_r[:, f, 0:512])
            # wq cols 0:256 (pairs 0,1) first
            for f4 in range(2):
                nc.sync.dma_start(out=wq_sb[:, 4 * f4:4 * f4 + 4, 0:256],
                                  in_=wqT_r[:, 4 * f4:4 * f4 + 4, 0:256])
            xq_t = {}
            for ck in range(4):
                xq_t[ck] = xqpool.tile([128, NF, 256], BF16,
                                       name=f"xq{ck}", tag="xq")
                for f4 in range(2):
                    nc.sync.dma_start(
                        out=xq_t[ck][:, 4 * f4:4 * f4 + 4, :],
                        in_=xqT_r[:, 4 * f4:4 * f4 + 4,
                                  ck * 256:(ck + 1) * 256])
            # xk block 1 immediately (K(p0,b1) gates ST(h0,kkt2))
            for f2 in range(4):
                nc.scalar.dma_start(
                    out=xk_sb[:, 2 * f2:2 * f2 + 2, 512:1024],
                    in_=xkT_r[:, 2 * f2:2 * f2 + 2, 512:1024])
            # mask chunk 0 + V inputs early (V(0) pulled ~slot 4)
            nm_t = {}
            def _nmload0(kkg):
                t = nmpool.tile([128, 2, 1024], BF16, name=f"nm0{kkg}",
                                tag="nm")
                nm_t[(0, kkg)] = t
                for half in range(2):
                    nc.scalar.dma_start(
                        out=t[:, half:half + 1, :],
                        in_=notmT_r[:, kkg * 2 + half:kkg * 2 + half + 1,
                                    0:1024])
            _nmload0(0)
            xv_t = {}
            for kkt in range(2):
                xv_t[kkt] = xvpool.tile([128, NF, 128], BF16,
                                        name=f"xv{kkt}", tag="xv")
                nc.scalar.dma_start(
                    out=xv_t[kkt][:],
                    in_=xvT_r[:, :, kkt * 128:(kkt + 1) * 128])
            for f2 in range(4):
                nc.sync.dma_start(
                    out=wv_sb[:, 2 * f2:2 * f2 + 2, :],
                    in_=wvT_r[:, 2 * f2:2 * f2 + 2, :])
            _nmload0(1)
            _nmload0(2)
            _nmload0(3)
            # wq cols 256:512 (pairs 2,3 - needed from h4)
            for f4 in range(2):
                nc.sync.dma_start(out=wq_sb[:, 4 * f4:4 * f4 + 4, 256:512],
                                  in_=wqT_r[:, 4 * f4:4 * f4 + 4, 256:512])
            for kkg in range(4, 8):
                _nmload0(kkg)
            # xk blocks 2-3, wo (bulk, later deadlines)
            for blk in range(2, 4):
                for f4 in range(2):
                    nc.scalar.dma_start(
                        out=xk_sb[:, 4 * f4:4 * f4 + 4,
                                  blk * 512:(blk + 1) * 512],
                        in_=xkT_r[:, 4 * f4:4 * f4 + 4,
                                  blk * 512:(blk + 1) * 512])
            for f4 in range(2):
                nc.scalar.dma_start(
                    out=wo_sb[:, 2 * f4:2 * f4 + 2, :],
                    in_=woT_r[:, 2 * f4:2 * f4 + 2, :])

            # ---------------- thunks (each ~1.7us of PE) ----------------
            def exps(shape_view):
                ps = stpool.tile([128, 1024], F32, name="xps")
                return ps, ps[:, 0:shape_view].rearrange("p a -> p a")

            def kproj_thunk(p, blk):
                def run():
                    if blk == 0 and p + 1 < NP:
                        wk_t[p + 1] = wkpool.tile([128, NF, 128], BF16,
                                                  name=f"wk{p+1}", tag="wk")
                        nc.sync.dma_start(
                            out=wk_t[p + 1][:],
                            in_=wkT_r[:, :, (p + 1) * 128:(p + 2) * 128])
                    psv = xpspool.tile([128, 512], F32, name="kps",
                                        tag="xps")
                    for fi in range(NF):
                        nc.tensor.matmul(
                            psv, wk_t[p][:, fi, :],
                            xk_sb[:, fi, blk * 512:(blk + 1) * 512],
                            start=(fi == 0), stop=(fi == NF - 1))
                    nc.vector.tensor_scalar(
                        out=kt[:, p, blk * 512:(blk + 1) * 512],
                        in0=psv, scalar1=bk_sb[:, p:p + 1],
                        scalar2=None, op0=Alu.add)
                return run

            def qproj_thunk(ck, pgrp):
                def run():
                    if pgrp == 1 and ck + 4 < 8:
                        nck = ck + 4
                        xq_t[nck] = xqpool.tile([128, NF, 256], BF16,
                                                name=f"xq{nck}", tag="xq")
                        nc.sync.dma_start(
                            out=xq_t[nck][:],
                            in_=xqT_r[:, :, nck * 256:(nck + 1) * 256])
                    for p in (2 * pgrp, 2 * pgrp + 1):
                        ps = xpspool.tile([128, 512], F32, name="qps",
                                          tag="xps")
                        psv = ps[:, 0:256]
                        for fi in range(NF):
                            nc.tensor.matmul(
                                psv, wq_sb[:, fi, p * 128:(p + 1) * 128],
                                xq_t[ck][:, fi, :],
                                start=(fi == 0), stop=(fi == NF - 1))
                        nc.vector.tensor_scalar(
                            out=qt[:, p, ck * 256:(ck + 1) * 256],
                            in0=psv, scalar1=0.125,
                            scalar2=bq8_sb[:, p:p + 1],
                            op0=Alu.mult, op1=Alu.add)
                return run

            def vproj_thunk(kkt):
                def run():
                    if kkt + 2 < NKT:
                        xv_t[kkt + 2] = xvpool.tile([128, NF, 128], BF16,
                                                    name=f"xv{kkt+2}",
                                                    tag="xv")
                        nc.scalar.dma_start(
                            out=xv_t[kkt + 2][:],
                            in_=xvT_r[:, :, (kkt + 2) * 128:(kkt + 3) * 128])
                    psv = xpspool.tile([128, 512], F32, name="vps",
                                        tag="xps")
                    for fi in range(NF):
                        nc.tensor.matmul(
                            psv, xv_t[kkt][:, fi, :], wv_sb[:, fi, :],
                            start=(fi == 0), stop=(fi == NF - 1))
                    nc.vector.tensor_copy(
                        out=vaug[:, kkt, :, 0:64],
                        in_=psv.rearrange("p (h d) -> p h d", d=64))
                    emitted_v.add(kkt)
                return run

            def nmload_thunk(kkg):
                def run():
                    t = nmpool.tile([128, 2, 1024], BF16, name=f"nm1{kkg}",
                                    tag="nm")
                    nm_t[(1, kkg)] = t
                    for half in range(2):
                        nc.sync.dma_start(
                            out=t[:, half:half + 1, :],
                            in_=notmT_r[:, kkg * 2 + half:
                                        kkg * 2 + half + 1, 1024:2048])
                return run

            def outproj_thunk(qti, et, alt=False):
                qoff = qti * 1024
                def run():
                    for qh in range(2):
                        if alt and qh == 1:
                            pst = ovpool.tile([128, 8, 128], F32,
                                              name="ops2", tag="ov")
                            psv = pst[:, 0:4, :].rearrange(
                                "p a b -> p (a b)")
                        else:
                            psv = xpspool.tile([128, 512], F32, name="ops",
                                               tag="xps")
                        for dc in range(NP):
                            nc.tensor.matmul(
                                psv, wo_sb[:, dc, et * 128:(et + 1) * 128],
                                ott[:, dc,
                                    qoff + qh * 512:qoff + (qh + 1) * 512],
                                start=(dc == 0), stop=(dc == NP - 1))
                        stg = ostg.tile([128, 512], F32, name="cstg")
                        nc.vector.tensor_scalar(
                            out=stg[:], in0=psv,
                            scalar1=bo2_sb[:, et:et + 1], scalar2=None,
                            op0=Alu.add)
                        for qq in range(2):
                            nc.sync.dma_start(
                                out=outT[et * 128:(et + 1) * 128,
                                         qoff + qh * 512 + qq * 256:
                                         qoff + qh * 512 + (qq + 1) * 256],
                                in_=stg[:, qq * 256:(qq + 1) * 256])
                return run

            emitted_v = set()

            def vproj_if_needed(kkt):
                if kkt not in emitted_v:
                    vproj_thunk(kkt)()

            # ---------------- upfront PE: minimum to start h0 ------------
            kproj_thunk(0, 0)()
            for ck in range(4):
                qproj_thunk(ck, 0)()

            # extras: {(qti,h): [(slot, thunk), ...]} (V is pulled on demand
            # by the PV drain, so only K/Q/nm/outproj are slotted here)
            X = {
                (0, 0): [(1, kproj_thunk(0, 1)), (5, kproj_thunk(0, 2)),
                         (9, kproj_thunk(0, 3))],
                (0, 1): [(0, kproj_thunk(1, 0)), (4, kproj_thunk(1, 1)),
                         (8, kproj_thunk(1, 2)), (12, kproj_thunk(1, 3))],
                (0, 2): [(2, kproj_thunk(2, 0)), (6, kproj_thunk(2, 1)),
                         (10, kproj_thunk(2, 2)), (14, kproj_thunk(2, 3))],
                (0, 3): [(1, qproj_thunk(0, 1)), (5, qproj_thunk(1, 1)),
                         (9, qproj_thunk(2, 1)), (13, qproj_thunk(3, 1))],
                (0, 4): [(1, kproj_thunk(3, 0)), (5, kproj_thunk(3, 1)),
                         (9, kproj_thunk(3, 2)), (13, kproj_thunk(3, 3))],
                (0, 5): [(1, qproj_thunk(4, 0)), (5, qproj_thunk(4, 1)),
                         (9, qproj_thunk(5, 0)), (13, qproj_thunk(5, 1)),
                         (11, nmload_thunk(0)), (15, nmload_thunk(1))],
                (0, 6): [(1, qproj_thunk(6, 0)), (5, qproj_thunk(6, 1)),
                         (3, nmload_thunk(2)), (7, nmload_thunk(3)),
                         (9, nmload_thunk(4)), (13, nmload_thunk(5))],
                (0, 7): [(1, qproj_thunk(7, 0)), (5, qproj_thunk(7, 1)),
                         (9, nmload_thunk(6)), (13, nmload_thunk(7))],
                (1, 0): [(2, outproj_thunk(0, 0)), (10, outproj_thunk(0, 1))],
                (1, 1): [(2, outproj_thunk(0, 2)), (10, outproj_thunk(0, 3))],
                (1, 2): [(2, outproj_thunk(0, 4)), (10, outproj_thunk(0, 5))],
                (1, 3): [(2, outproj_thunk(0, 6)), (10, outproj_thunk(0, 7))],
            }

            # ---------------- spine with pending-PV queue ----------------
            pvq = []   # entries: [qti, h, kkt, pt, ov, remaining-first-flag]
            seg_left = {}  # (qti,h) -> count of unemitted PVs

            def head_finish(qti, h, ov, otn):
                p, lo = h // 2, (h % 2) * 64
                qoff = qti * 1024
                onorm = onpool.tile([128, 8, 65], F32, name="onorm")
                nc.vector.tensor_copy(out=onorm[:], in_=ov[:, :, 0:65])
                for qs in range(8):
                    nc.gpsimd.normalize_recip(
                        otn[:, qs, lo:lo + 64],
                        onorm[:, qs, 0:64],
                        onorm[:, qs, 64:65])
                if h % 2 == 1:
                    for qs in range(8):
                        nc.sync.dma_start_transpose(
                            ott[:, p,
                                qoff + qs * 128:qoff + (qs + 1) * 128],
                            otn[:, qs, :])

            def emit_one_pv():
                qti, h, kkt, ptt, ov, otn = pvq.pop(0)
                for qs in range(8):
                    nc.tensor.matmul(
                        ov[:, qs, 0:65],
                        ptt[:, qs * 128:(qs + 1) * 128],
                        vaug[:, kkt, h, :],
                        start=(kkt == 0 and qs % 4 == 0),
                        stop=(kkt == NKT - 1 and qs % 4 == 3),
                        skip_group_check=qs % 4 not in (0, 3))
                seg_left[(qti, h)] -= 1
                if seg_left[(qti, h)] == 0:
                    head_finish(qti, h, ov, otn)

            def drain_pv(limit_pending):
                while len(pvq) > limit_pending:
                    vproj_if_needed(pvq[0][2])
                    emit_one_pv()

            otn = None
            for qti in range(2):
                qoff = qti * 1024
                for h in range(8):
                    p, lo = h // 2, (h % 2) * 64
                    slots = [[] for _ in range(16)]
                    for s, th in X.get((qti, h), []):
                        slots[s].append(th)
                    ov = ovpool.tile([128, 8, 128], F32, name="ov", tag="ov")
                    if h % 2 == 0:
                        otn = otnpool.tile([128, 8, 128], BF16,
                                           name="otn", tag="otn")
                    seg_left[(qti, h)] = NKT
                    lag = 0 if (qti, h) == (1, 7) else LAG
                    for kkt in range(NKT):
                        drain_pv(lag)
                        st = stpool.tile([128, 1024], F32, name="st", tag="st")
                        for j in range(2):
                            nc.tensor.matmul(
                                st[:, j * 512:(j + 1) * 512],
                                kt[lo:lo + 64, p, kkt * 128:(kkt + 1) * 128],
                                qt[lo:lo + 64, p,
                                   qoff + j * 512:qoff + (j + 1) * 512],
                                start=True, stop=True,
                                tile_position=(lo, 0))
                        pt = ptpool.tile([128, 1024], BF16, name="pt")
                        nc.scalar.activation(pt[:], st[:], Act.Exp)
                        nmt = nm_t[(qti, kkt // 2)]
                        nc.vector.tensor_tensor(
                            out=pt[:], in0=pt[:],
                            in1=nmt[:, kkt % 2, :], op=Alu.mult)
                        pvq.append([qti, h, kkt, pt, ov, otn])
                        for th in slots[kkt]:
                            th()
                    drain_pv(0)
            # tail: q-tile-1 output projection (extra PSUM depth via the
            # now-idle ov pool keeps the PE fed and at full p-state)
            for et in range(NF):
                outproj_thunk(1, et, alt=True)()
    nc.compile()
    return nc


def _get_nc():
    if "nc" not in _STATE:
        _STATE["nc"] = build_nc()
    return _STATE["nc"]


def kernel(query, key, value, mask, Wq, bq, Wk, bk, Wv, bv, Wo, bo):
    query = np.asarray(query, dtype=np.float32)
    key = np.asarray(key, dtype=np.float32)
    value = np.asarray(value, dtype=np.float32)
    mask = np.asarray(mask)
    Wq = np.asarray(Wq, dtype=np.float32)
    Wk = np.asarray(Wk, dtype=np.float32)
    Wv = np.asarray(Wv, dtype=np.float32)
    Wo = np.asarray(Wo, dtype=np.float32)
    bq = np.asarray(bq, dtype=np.float32)
    bk = np.asarray(bk, dtype=np.float32)
    bv = np.asarray(bv, dtype=np.float32)
    bo = np.asarray(bo, dtype=np.float32)

    bf = ml_dtypes.bfloat16
    xq_b, xk_b, xv_b, nm_b = [], [], [], []
    for b in range(4):
        xq_b.append(np.ascontiguousarray(query[b].T).astype(bf))
        xk_b.append(np.ascontiguousarray(key[b].T).astype(bf))
        xv_b.append(np.ascontiguousarray(value[b].T).astype(bf))
        nm_b.append(np.ascontiguousarray(
            (~mask[b, 0]).T.astype(bf)))
    wq_g, wk_g, wv_g, wo_g, bq_g, bk_g, bo_g = [], [], [], [], [], [], []
    for hg in range(2):
        dsl = slice(hg * 512, (hg + 1) * 512)
        wq_g.append(np.ascontiguousarray(Wq.T[:, dsl]).astype(bf))
        wk_g.append(np.ascontiguousarray(Wk.T[:, dsl]).astype(bf))
        wv_g.append(np.ascontiguousarray(Wv.T[:, dsl]).astype(bf))
        wo_g.append(np.ascontiguousarray(Wo[:, dsl].T).astype(bf))
        bq_g.append(np.ascontiguousarray((bq[dsl] / 8.0).reshape(NP, 128).T))
        bk_g.append(np.ascontiguousarray(bk[dsl].reshape(NP, 128).T))
        bo_g.append(np.ascontiguousarray(
            (bo / 2.0 + Wo[:, dsl] @ bv[dsl]).reshape(NF, 128).T))

    in_maps = []
    for c in range(N_CORES):
        b, hg = c // 2, c % 2
        in_maps.append({
            "xqT": xq_b[b], "xkT": xk_b[b], "xvT": xv_b[b],
            "wqT": wq_g[hg], "wkT": wk_g[hg], "wvT": wv_g[hg],
            "woT": wo_g[hg],
            "bq8": bq_g[hg], "bk_l": bk_g[hg], "bo2": bo_g[hg],
            "notmT": nm_b[b],
        })

    nc = _get_nc()
    res = run_bass_kernel_spmd(nc, in_maps, core_ids=list(range(N_CORES)))
    out = np.empty((4, SQ, EMBED), dtype=np.float32)
    for b in range(4):
        acc = res.results[2 * b]["outT"] + res.results[2 * b + 1]["outT"]
        out[b] = acc.T
    return out


# revision 6
# speedup vs baseline: 1.2254x; 1.0111x over previous
"""MultiHeadAttention TRN2 kernel, 8-core SPMD — interleaved emission (V3.1).

Sharding: core c -> batch b=c//2, head-group hg=c%2 (8 heads / 512 dims).
All inputs bf16 except biases. Spine is q-tile-major attention; K/Q/V
projections and the q-tile-0 output projection are "extras" interleaved
into the PE stream so the ACT engine (exp, the ~266us floor) stays fed.
PV matmuls trail the ST/exp stream through a pending queue (P stationary,
free=65 -> half PE cost) and may slip across segment boundaries.

PSUM accumulation groups are per 2KB zero region (bank): start zeroes the
whole bank -> one start/stop per bank, interior matmuls skip group check.
Extras borrow PSUM from the st pool (ov pool slots live too long).
"""
import numpy as np
import ml_dtypes

import concourse.bass as bass
import concourse.mybir as mybir
import concourse.tile as tile
from concourse import bacc
from concourse.bass_utils import run_bass_kernel_spmd

F32 = mybir.dt.float32
BF16 = mybir.dt.bfloat16
Act = mybir.ActivationFunctionType
Alu = mybir.AluOpType

EMBED = 1024
SK = 2048
SQ = 2048
NF = 8
NKT = 16
NP = 4
N_CORES = 8
LAG = 3

_STATE = {}


def build_nc():
    nc = bacc.Bacc("TRN2", target_bir_lowering=False)
    xqT = nc.dram_tensor("xqT", [EMBED, SQ], BF16, kind="ExternalInput")
    xkT = nc.dram_tensor("xkT", [EMBED, SK], BF16, kind="ExternalInput")
    xvT = nc.dram_tensor("xvT", [EMBED, SK], BF16, kind="ExternalInput")
    wqT = nc.dram_tensor("wqT", [EMBED, 512], BF16, kind="ExternalInput")
    wkT = nc.dram_tensor("wkT", [EMBED, 512], BF16, kind="ExternalInput")
    wvT = nc.dram_tensor("wvT", [EMBED, 512], BF16, kind="ExternalInput")
    woT = nc.dram_tensor("woT", [512, EMBED], BF16, kind="ExternalInput")
    bq8 = nc.dram_tensor("bq8", [128, NP], F32, kind="ExternalInput")
    bk_l = nc.dram_tensor("bk_l", [128, NP], F32, kind="ExternalInput")
    bo2 = nc.dram_tensor("bo2", [128, NF], F32, kind="ExternalInput")
    notmT = nc.dram_tensor("notmT", [SK, SQ], BF16, kind="ExternalInput")
    outT = nc.dram_tensor("outT", [EMBED, SQ], F32, kind="ExternalOutput")

    xqT_r = xqT.rearrange("(t p) q -> p t q", p=128)
    xkT_r = xkT.rearrange("(t p) k -> p t k", p=128)
    xvT_r = xvT.rearrange("(t p) k -> p t k", p=128)
    wqT_r = wqT.rearrange("(t p) n -> p t n", p=128)
    wkT_r = wkT.rearrange("(t p) n -> p t n", p=128)
    wvT_r = wvT.rearrange("(t p) n -> p t n", p=128)
    woT_r = woT.rearrange("(t p) n -> p t n", p=128)
    notmT_r = notmT.rearrange("(t p) q -> p t q", p=128)

    with tile.TileContext(nc) as tc:
        with tc.tile_pool(name="persist", bufs=1) as pp, \
             tc.tile_pool(name="bias", bufs=1) as bp, \
             tc.tile_pool(name="nmch", bufs=8) as nmpool, \
             tc.tile_pool(name="wkch", bufs=2) as wkpool, \
             tc.tile_pool(name="xqch", bufs=4) as xqpool, \
             tc.tile_pool(name="xvch", bufs=2) as xvpool, \
             tc.tile_pool(name="apt", bufs=7) as ptpool, \
             tc.tile_pool(name="aon", bufs=1) as onpool, \
             tc.tile_pool(name="aotn", bufs=2) as otnpool, \
             tc.tile_pool(name="aost", bufs=3) as ostg, \
             tc.tile_pool(name="ast", bufs=2, space="PSUM") as stpool, \
             tc.tile_pool(name="axps", bufs=2, space="PSUM") as xpspool, \
             tc.tile_pool(name="aov", bufs=1, space="PSUM") as ovpool:
            kt = pp.tile([128, NP, SK], BF16, name="kt")
            qt = pp.tile([128, NP, SQ], BF16, name="qt")
            vaug = pp.tile([128, NKT, 8, 65], BF16, name="vaug")
            xk_sb = pp.tile([128, NF, SK], BF16, name="xk_sb")
            wq_sb = pp.tile([128, NF, 512], BF16, name="wq_sb")
            wv_sb = pp.tile([128, NF, 512], BF16, name="wv_sb")
            wo_sb = pp.tile([128, NP, EMBED], BF16, name="wo_sb")
            ott = pp.tile([128, NP, SQ], BF16, name="ott")
            bq8_sb = bp.tile([128, NP], F32, name="bq8_sb")
            bk_sb = bp.tile([128, NP], F32, name="bk_sb")
            bo2_sb = bp.tile([128, NF], F32, name="bo2_sb")
            nc.vector.memset(vaug[:, :, :, 64:65], 1.0)

            # ---------------- DMA preamble (urgency-ordered) ----------
            nc.sync.dma_start(out=bk_sb[:], in_=bk_l[:, :])
            nc.sync.dma_start(out=bq8_sb[:], in_=bq8[:, :])
            nc.sync.dma_start(out=bo2_sb[:], in_=bo2[:, :])
            wk_t = {}
            wk_t[0] = wkpool.tile([128, NF, 128], BF16, name="wk0", tag="wk")
            for f4 in range(2):
                nc.sync.dma_start(out=wk_t[0][:, 4 * f4:4 * f4 + 4, :],
                                  in_=wkT_r[:, 4 * f4:4 * f4 + 4, 0:128])
            # xk block 0 fine-grained (feeds K(p0,b0) asap)
            for f in range(NF):
                nc.scalar.dma_start(out=xk_sb[:, f, 0:512],
                                    in_=xkT_r[:, f, 0:512])
            # wq cols 0:256 (pairs 0,1) first
            for f4 in range(2):
                nc.sync.dma_start(out=wq_sb[:, 4 * f4:4 * f4 + 4, 0:256],
                                  in_=wqT_r[:, 4 * f4:4 * f4 + 4, 0:256])
            xq_t = {}
            for ck in range(4):
                xq_t[ck] = xqpool.tile([128, NF, 256], BF16,
                                       name=f"xq{ck}", tag="xq")
                for f4 in range(2):
                    nc.sync.dma_start(
                        out=xq_t[ck][:, 4 * f4:4 * f4 + 4, :],
                        in_=xqT_r[:, 4 * f4:4 * f4 + 4,
                                  ck * 256:(ck + 1) * 256])
            # xk block 1 immediately (K(p0,b1) gates ST(h0,kkt2))
            for f2 in range(4):
                nc.scalar.dma_start(
                    out=xk_sb[:, 2 * f2:2 * f2 + 2, 512:1024],
                    in_=xkT_r[:, 2 * f2:2 * f2 + 2, 512:1024])
            # mask chunk 0 + V inputs early (V(0) pulled ~slot 4)
            nm_t = {}
            def _nmload0(kkg):
                t = nmpool.tile([128, 2, 1024], BF16, name=f"nm0{kkg}",
                                tag="nm")
                nm_t[(0, kkg)] = t
                for half in range(2):
                    nc.scalar.dma_start(
                        out=t[:, half:half + 1, :],
                        in_=notmT_r[:, kkg * 2 + half:kkg * 2 + half + 1,
                                    0:1024])
            _nmload0(0)
            xv_t = {}
            for kkt in range(2):
                xv_t[kkt] = xvpool.tile([128, NF, 128], BF16,
                                        name=f"xv{kkt}", tag="xv")
                nc.scalar.dma_start(
                    out=xv_t[kkt][:],
                    in_=xvT_r[:, :, kkt * 128:(kkt + 1) * 128])
            for f2 in range(4):
                nc.sync.dma_start(
                    out=wv_sb[:, 2 * f2:2 * f2 + 2, :],
                    in_=wvT_r[:, 2 * f2:2 * f2 + 2, :])
            _nmload0(1)
            _nmload0(2)
            _nmload0(3)
            # wq cols 256:512 (pairs 2,3 - needed from h4)
            for f4 in range(2):
                nc.sync.dma_start(out=wq_sb[:, 4 * f4:4 * f4 + 4, 256:512],
                                  in_=wqT_r[:, 4 * f4:4 * f4 + 4, 256:512])
            for kkg in range(4, 8):
                _nmload0(kkg)
            # xk blocks 2-3, wo (bulk, later deadlines)
            for blk in range(2, 4):
                for f4 in range(2):
                    nc.scalar.dma_start(
                        out=xk_sb[:, 4 * f4:4 * f4 + 4,
                                  blk * 512:(blk + 1) * 512],
                        in_=xkT_r[:, 4 * f4:4 * f4 + 4,
                                  blk * 512:(blk + 1) * 512])
            for f4 in range(2):
                nc.scalar.dma_start(
                    out=wo_sb[:, 2 * f4:2 * f4 + 2, :],
                    in_=woT_r[:, 2 * f4:2 * f4 + 2, :])

            # ---------------- thunks (each ~1.7us of PE) ----------------
            def exps(shape_view):
                ps = stpool.tile([128, 1024], F32, name="xps")
                return ps, ps[:, 0:shape_view].rearrange("p a -> p a")

            def kproj_thunk(p, blk):
                def run():
                    if blk == 0 and p + 1 < NP:
                        wk_t[p + 1] = wkpool.tile([128, NF, 128], BF16,
                                                  name=f"wk{p+1}", tag="wk")
                        nc.sync.dma_start(
                            out=wk_t[p + 1][:],
                            in_=wkT_r[:, :, (p + 1) * 128:(p + 2) * 128])
                    psv = xpspool.tile([128, 512], F32, name="kps",
                                        tag="xps")
                    for fi in range(NF):
                        nc.tensor.matmul(
                            psv, wk_t[p][:, fi, :],
                            xk_sb[:, fi, blk * 512:(blk + 1) * 512],
                            start=(fi == 0), stop=(fi == NF - 1))
                    nc.vector.tensor_scalar(
                        out=kt[:, p, blk * 512:(blk + 1) * 512],
                        in0=psv, scalar1=bk_sb[:, p:p + 1],
                        scalar2=None, op0=Alu.add)
                return run

            def qproj_thunk(ck, pgrp):
                def run():
                    if pgrp == 1 and ck + 4 < 8:
                        nck = ck + 4
                        xq_t[nck] = xqpool.tile([128, NF, 256], BF16,
                                                name=f"xq{nck}", tag="xq")
                        nc.sync.dma_start(
                            out=xq_t[nck][:],
                            in_=xqT_r[:, :, nck * 256:(nck + 1) * 256])
                    for p in (2 * pgrp, 2 * pgrp + 1):
                        ps = xpspool.tile([128, 512], F32, name="qps",
                                          tag="xps")
                        psv = ps[:, 0:256]
                        for fi in range(NF):
                            nc.tensor.matmul(
                                psv, wq_sb[:, fi, p * 128:(p + 1) * 128],
                                xq_t[ck][:, fi, :],
                                start=(fi == 0), stop=(fi == NF - 1))
                        nc.vector.tensor_scalar(
                            out=qt[:, p, ck * 256:(ck + 1) * 256],
                            in0=psv, scalar1=0.125,
                            scalar2=bq8_sb[:, p:p + 1],
                            op0=Alu.mult, op1=Alu.add)
                return run

            def vproj_thunk(kkt):
                def run():
                    if kkt + 2 < NKT:
                        xv_t[kkt + 2] = xvpool.tile([128, NF, 128], BF16,
                                                    name=f"xv{kkt+2}",
                                                    tag="xv")
                        nc.scalar.dma_start(
                            out=xv_t[kkt + 2][:],
                            in_=xvT_r[:, :, (kkt + 2) * 128:(kkt + 3) * 128])
                    psv = xpspool.tile([128, 512], F32, name="vps",
                                        tag="xps")
                    for fi in range(NF):
                        nc.tensor.matmul(
                            psv, xv_t[kkt][:, fi, :], wv_sb[:, fi, :],
                            start=(fi == 0), stop=(fi == NF - 1))
                    nc.vector.tensor_copy(
                        out=vaug[:, kkt, :, 0:64],
                        in_=psv.rearrange("p (h d) -> p h d", d=64))
                    emitted_v.add(kkt)
                return run

            def nmload_thunk(kkg):
                def run():
                    t = nmpool.tile([128, 2, 1024], BF16, name=f"nm1{kkg}",
                                    tag="nm")
                    nm_t[(1, kkg)] = t
                    for half in range(2):
                        nc.sync.dma_start(
                            out=t[:, half:half + 1, :],
                            in_=notmT_r[:, kkg * 2 + half:
                                        kkg * 2 + half + 1, 1024:2048])
                return run

            def outproj_thunk(qti, et, alt=False):
                qoff = qti * 1024
                def run():
                    for qh in range(2):
                        if alt and qh == 1:
                            pst = ovpool.tile([128, 8, 128], F32,
                                              name="ops2", tag="ov")
                            psv = pst[:, 0:4, :].rearrange(
                                "p a b -> p (a b)")
                        else:
                            psv = xpspool.tile([128, 512], F32, name="ops",
                                               tag="xps")
                        for dc in range(NP):
                            nc.tensor.matmul(
                                psv, wo_sb[:, dc, et * 128:(et + 1) * 128],
                                ott[:, dc,
                                    qoff + qh * 512:qoff + (qh + 1) * 512],
                                start=(dc == 0), stop=(dc == NP - 1))
                        stg = ostg.tile([128, 512], F32, name="cstg")
                        nc.vector.tensor_scalar(
                            out=stg[:], in0=psv,
                            scalar1=bo2_sb[:, et:et + 1], scalar2=None,
                            op0=Alu.add)
                        for qq in range(2):
                            nc.sync.dma_start(
                                out=outT[et * 128:(et + 1) * 128,
                                         qoff + qh * 512 + qq * 256:
                                         qoff + qh * 512 + (qq + 1) * 256],
                                in_=stg[:, qq * 256:(qq + 1) * 256])
                return run

            emitted_v = set()

            def vproj_if_needed(kkt):
                if kkt not in emitted_v:
                    vproj_thunk(kkt)()

            # ---------------- upfront PE: minimum to start h0 ------------
            kproj_thunk(0, 0)()
            for ck in range(4):
                qproj_thunk(ck, 0)()

            # extras: {(qti,h): [(slot, thunk), ...]} (V is pulled on demand
            # by the PV drain, so only K/Q/nm/outproj are slotted here)
            X = {
                (0, 0): [(1, kproj_thunk(0, 1)), (5, kproj_thunk(0, 2)),
                         (9, kproj_thunk(0, 3))],
                (0, 1): [(0, kproj_thunk(1, 0)), (4, kproj_thunk(1, 1)),
                         (8, kproj_thunk(1, 2)), (12, kproj_thunk(1, 3))],
                (0, 2): [(2, kproj_thunk(2, 0)), (6, kproj_thunk(2, 1)),
                         (10, kproj_thunk(2, 2)), (14, kproj_thunk(2, 3))],
                (0, 3): [(1, qproj_thunk(0, 1)), (5, qproj_thunk(1, 1)),
                         (9, qproj_thunk(2, 1)), (13, qproj_thunk(3, 1))],
                (0, 4): [(1, kproj_thunk(3, 0)), (5, kproj_thunk(3, 1)),
                         (9, kproj_thunk(3, 2)), (13, kproj_thunk(3, 3))],
                (0, 5): [(1, qproj_thunk(4, 0)), (5, qproj_thunk(4, 1)),
                         (9, qproj_thunk(5, 0)), (13, qproj_thunk(5, 1)),
                         (11, nmload_thunk(0)), (15, nmload_thunk(1))],
                (0, 6): [(1, qproj_thunk(6, 0)), (5, qproj_thunk(6, 1)),
                         (3, nmload_thunk(2)), (7, nmload_thunk(3)),
                         (9, nmload_thunk(4)), (13, nmload_thunk(5))],
                (0, 7): [(1, qproj_thunk(7, 0)), (5, qproj_thunk(7, 1)),
                         (9, nmload_thunk(6)), (13, nmload_thunk(7))],
                (1, 0): [(2, outproj_thunk(0, 0)), (10, outproj_thunk(0, 1))],
                (1, 1): [(2, outproj_thunk(0, 2)), (10, outproj_thunk(0, 3))],
                (1, 2): [(2, outproj_thunk(0, 4)), (10, outproj_thunk(0, 5))],
                (1, 3): [(2, outproj_thunk(0, 6)), (10, outproj_thunk(0, 7))],
            }

            # ---------------- spine with pending-PV queue ----------------
            pvq = []   # entries: [qti, h, kkt, pt, ov, remaining-first-flag]
            seg_left = {}  # (qti,h) -> count of unemitted PVs

            def head_finish(qti, h, ov, otn):
                p, lo = h // 2, (h % 2) * 64
                qoff = qti * 1024
                onorm = onpool.tile([128, 8, 65], F32, name="onorm")
                nc.vector.tensor_copy(out=onorm[:], in_=ov[:, :, 0:65])
                for qs in range(8):
                    nc.gpsimd.normalize_recip(
                        otn[:, qs, lo:lo + 64],
                        onorm[:, qs, 0:64],
                        onorm[:, qs, 64:65])
                if h % 2 == 1:
                    for qs in range(8):
                        nc.sync.dma_start_transpose(
                            ott[:, p,
                                qoff + qs * 128:qoff + (qs + 1) * 128],
                            otn[:, qs, :])

            def emit_one_pv():
                qti, h, kkt, ptt, ov, otn = pvq.pop(0)
                for qs in range(8):
                    nc.tensor.matmul(
                        ov[:, qs, 0:65],
                        ptt[:, qs * 128:(qs + 1) * 128],
                        vaug[:, kkt, h, :],
                        start=(kkt == 0 and qs % 4 == 0),
                        stop=(kkt == NKT - 1 and qs % 4 == 3),
                        skip_group_check=qs % 4 not in (0, 3))
                seg_left[(qti, h)] -= 1
                if seg_left[(qti, h)] == 0:
                    head_finish(qti, h, ov, otn)

            def drain_pv(limit_pending):
                while len(pvq) > limit_pending:
                    vproj_if_needed(pvq[0][2])
                    emit_one_pv()

            otn = None
            for qti in range(2):
                qoff = qti * 1024
                for h in range(8):
                    p, lo = h // 2, (h % 2) * 64
                    slots = [[] for _ in range(16)]
                    for s, th in X.get((qti, h), []):
                        slots[s].append(th)
                    ov = ovpool.tile([128, 8, 128], F32, name="ov", tag="ov")
                    if h % 2 == 0:
                        otn = otnpool.tile([128, 8, 128], BF16,
                                           name="otn", tag="otn")
                    seg_left[(qti, h)] = NKT
                    lag = 0 if (qti, h) == (1, 7) else LAG
                    for kkt in range(NKT):
                        drain_pv(lag)
                        with tc.high_priority():
                            st = stpool.tile([128, 1024], F32, name="st",
                                             tag="st")
                            for j in range(2):
                                nc.tensor.matmul(
                                    st[:, j * 512:(j + 1) * 512],
                                    kt[lo:lo + 64, p,
                                       kkt * 128:(kkt + 1) * 128],
                                    qt[lo:lo + 64, p,
                                       qoff + j * 512:qoff + (j + 1) * 512],
                                    start=True, stop=True,
                                    tile_position=(lo, 0))
                            pt = ptpool.tile([128, 1024], BF16, name="pt")
                            nc.scalar.activation(pt[:], st[:], Act.Exp)
                            nmt = nm_t[(qti, kkt // 2)]
                            nc.vector.tensor_tensor(
                                out=pt[:], in0=pt[:],
                                in1=nmt[:, kkt % 2, :], op=Alu.mult)
                        pvq.append([qti, h, kkt, pt, ov, otn])
                        for th in slots[kkt]:
                            th()
                    drain_pv(0)
            # tail: q-tile-1 output projection (extra PSUM depth via the
            # now-idle ov pool keeps the PE fed and at full p-state)
            for et in range(NF):
                outproj_thunk(1, et, alt=True)()
    nc.compile()
    return nc


def _get_nc():
    if "nc" not in _STATE:
        _STATE["nc"] = build_nc()
    return _STATE["nc"]


def kernel(query, key, value, mask, Wq, bq, Wk, bk, Wv, bv, Wo, bo):
    query = np.asarray(query, dtype=np.float32)
    key = np.asarray(key, dtype=np.float32)
    value = np.asarray(value, dtype=np.float32)
    mask = np.asarray(mask)
    Wq = np.asarray(Wq, dtype=np.float32)
    Wk = np.asarray(Wk, dtype=np.float32)
    Wv = np.asarray(Wv, dtype=np.float32)
    Wo = np.asarray(Wo, dtype=np.float32)
    bq = np.asarray(bq, dtype=np.float32)
    bk = np.asarray(bk, dtype=np.float32)
    bv = np.asarray(bv, dtype=np.float32)
    bo = np.asarray(bo, dtype=np.float32)

    bf = ml_dtypes.bfloat16
    xq_b, xk_b, xv_b, nm_b = [], [], [], []
    for b in range(4):
        xq_b.append(np.ascontiguousarray(query[b].T).astype(bf))
        xk_b.append(np.ascontiguousarray(key[b].T).astype(bf))
        xv_b.append(np.ascontiguousarray(value[b].T).astype(bf))
        nm_b.append(np.ascontiguousarray(
            (~mask[b, 0]).T.astype(bf)))
    wq_g, wk_g, wv_g, wo_g, bq_g, bk_g, bo_g = [], [], [], [], [], [], []
    for hg in range(2):
        dsl = slice(hg * 512, (hg + 1) * 512)
        wq_g.append(np.ascontiguousarray(Wq.T[:, dsl]).astype(bf))
        wk_g.append(np.ascontiguousarray(Wk.T[:, dsl]).astype(bf))
        wv_g.append(np.ascontiguousarray(Wv.T[:, dsl]).astype(bf))
        wo_g.append(np.ascontiguousarray(Wo[:, dsl].T).astype(bf))
        bq_g.append(np.ascontiguousarray((bq[dsl] / 8.0).reshape(NP, 128).T))
        bk_g.append(np.ascontiguousarray(bk[dsl].reshape(NP, 128).T))
        bo_g.append(np.ascontiguousarray(
            (bo / 2.0 + Wo[:, dsl] @ bv[dsl]).reshape(NF, 128).T))

    in_maps = []
    for c in range(N_CORES):
        b, hg = c // 2, c % 2
        in_maps.append({
            "xqT": xq_b[b], "xkT": xk_b[b], "xvT": xv_b[b],
            "wqT": wq_g[hg], "wkT": wk_g[hg], "wvT": wv_g[hg],
            "woT": wo_g[hg],
            "bq8": bq_g[hg], "bk_l": bk_g[hg], "bo2": bo_g[hg],
            "notmT": nm_b[b],
        })

    nc = _get_nc()
    res = run_bass_kernel_spmd(nc, in_maps, core_ids=list(range(N_CORES)))
    out = np.empty((4, SQ, EMBED), dtype=np.float32)
    for b in range(4):
        acc = res.results[2 * b]["outT"] + res.results[2 * b + 1]["outT"]
        out[b] = acc.T
    return out


# revision 8
# speedup vs baseline: 1.2422x; 1.0137x over previous
"""MultiHeadAttention TRN2 kernel, 8-core SPMD — interleaved emission (V3.1).

Sharding: core c -> batch b=c//2, head-group hg=c%2 (8 heads / 512 dims).
All inputs bf16 except biases. Spine is q-tile-major attention; K/Q/V
projections and the q-tile-0 output projection are "extras" interleaved
into the PE stream so the ACT engine (exp, the ~266us floor) stays fed.
PV matmuls trail the ST/exp stream through a pending queue (P stationary,
free=65 -> half PE cost) and may slip across segment boundaries.

PSUM accumulation groups are per 2KB zero region (bank): start zeroes the
whole bank -> one start/stop per bank, interior matmuls skip group check.
Extras borrow PSUM from the st pool (ov pool slots live too long).
"""
import numpy as np
import ml_dtypes

import concourse.bass as bass
import concourse.mybir as mybir
import concourse.tile as tile
from concourse import bacc
from concourse.bass_utils import run_bass_kernel_spmd

F32 = mybir.dt.float32
BF16 = mybir.dt.bfloat16
Act = mybir.ActivationFunctionType
Alu = mybir.AluOpType

EMBED = 1024
SK = 2048
SQ = 2048
NF = 8
NKT = 16
NP = 4
N_CORES = 8
LAG = 3

_STATE = {}


def build_nc():
    nc = bacc.Bacc("TRN2", target_bir_lowering=False)
    xqT = nc.dram_tensor("xqT", [EMBED, SQ], BF16, kind="ExternalInput")
    xkT = nc.dram_tensor("xkT", [EMBED, SK], BF16, kind="ExternalInput")
    xvT = nc.dram_tensor("xvT", [EMBED, SK], BF16, kind="ExternalInput")
    wqT = nc.dram_tensor("wqT", [EMBED, 512], BF16, kind="ExternalInput")
    wkT = nc.dram_tensor("wkT", [EMBED, 512], BF16, kind="ExternalInput")
    wvT = nc.dram_tensor("wvT", [EMBED, 512], BF16, kind="ExternalInput")
    woT = nc.dram_tensor("woT", [512, EMBED], BF16, kind="ExternalInput")
    bq8 = nc.dram_tensor("bq8", [128, NP], F32, kind="ExternalInput")
    bk_l = nc.dram_tensor("bk_l", [128, NP], F32, kind="ExternalInput")
    bo2 = nc.dram_tensor("bo2", [128, NF], F32, kind="ExternalInput")
    notmT = nc.dram_tensor("notmT", [SK, SQ], BF16, kind="ExternalInput")
    outT = nc.dram_tensor("outT", [EMBED, SQ], F32, kind="ExternalOutput")

    xqT_r = xqT.rearrange("(t p) q -> p t q", p=128)
    xkT_r = xkT.rearrange("(t p) k -> p t k", p=128)
    xvT_r = xvT.rearrange("(t p) k -> p t k", p=128)
    wqT_r = wqT.rearrange("(t p) n -> p t n", p=128)
    wkT_r = wkT.rearrange("(t p) n -> p t n", p=128)
    wvT_r = wvT.rearrange("(t p) n -> p t n", p=128)
    woT_r = woT.rearrange("(t p) n -> p t n", p=128)
    notmT_r = notmT.rearrange("(t p) q -> p t q", p=128)

    with tile.TileContext(nc) as tc:
        with tc.tile_pool(name="persist", bufs=1) as pp, \
             tc.tile_pool(name="bias", bufs=1) as bp, \
             tc.tile_pool(name="nmch", bufs=8) as nmpool, \
             tc.tile_pool(name="wkch", bufs=2) as wkpool, \
             tc.tile_pool(name="xqch", bufs=4) as xqpool, \
             tc.tile_pool(name="xvch", bufs=2) as xvpool, \
             tc.tile_pool(name="apt", bufs=7) as ptpool, \
             tc.tile_pool(name="aon", bufs=1) as onpool, \
             tc.tile_pool(name="aotn", bufs=2) as otnpool, \
             tc.tile_pool(name="aost", bufs=3) as ostg, \
             tc.tile_pool(name="ast", bufs=2, space="PSUM") as stpool, \
             tc.tile_pool(name="axps", bufs=1, space="PSUM") as xpspool, \
             tc.tile_pool(name="aov", bufs=3, space="PSUM") as ovpool:
            kt = pp.tile([128, NP, SK], BF16, name="kt")
            qt = pp.tile([128, NP, SQ], BF16, name="qt")
            vaug = pp.tile([128, NKT, 8, 65], BF16, name="vaug")
            xk_sb = pp.tile([128, NF, SK], BF16, name="xk_sb")
            wq_sb = pp.tile([128, NF, 512], BF16, name="wq_sb")
            wv_sb = pp.tile([128, NF, 512], BF16, name="wv_sb")
            wo_sb = pp.tile([128, NP, EMBED], BF16, name="wo_sb")
            ott = pp.tile([128, NP, SQ], BF16, name="ott")
            bq8_sb = bp.tile([128, NP], F32, name="bq8_sb")
            bk_sb = bp.tile([128, NP], F32, name="bk_sb")
            bo2_sb = bp.tile([128, NF], F32, name="bo2_sb")
            nc.vector.memset(vaug[:, :, :, 64:65], 1.0)

            # ---------------- DMA preamble (urgency-ordered) ----------
            nc.sync.dma_start(out=bk_sb[:], in_=bk_l[:, :])
            nc.sync.dma_start(out=bq8_sb[:], in_=bq8[:, :])
            nc.sync.dma_start(out=bo2_sb[:], in_=bo2[:, :])
            wk_t = {}
            wk_t[0] = wkpool.tile([128, NF, 128], BF16, name="wk0", tag="wk")
            for f4 in range(2):
                nc.sync.dma_start(out=wk_t[0][:, 4 * f4:4 * f4 + 4, :],
                                  in_=wkT_r[:, 4 * f4:4 * f4 + 4, 0:128])
            # xk block 0 fine-grained (feeds K(p0,b0) asap)
            for f2 in range(4):
                nc.scalar.dma_start(out=xk_sb[:, 2 * f2:2 * f2 + 2, 0:512],
                                    in_=xkT_r[:, 2 * f2:2 * f2 + 2, 0:512])
            # wq cols 0:256 (pairs 0,1) first
            for f4 in range(2):
                nc.sync.dma_start(out=wq_sb[:, 4 * f4:4 * f4 + 4, 0:256],
                                  in_=wqT_r[:, 4 * f4:4 * f4 + 4, 0:256])
            xq_t = {}
            for ck in range(4):
                xq_t[ck] = xqpool.tile([128, NF, 256], BF16,
                                       name=f"xq{ck}", tag="xq")
                nc.sync.dma_start(
                    out=xq_t[ck][:],
                    in_=xqT_r[:, :, ck * 256:(ck + 1) * 256])
            # xk block 1 immediately (K(p0,b1) gates ST(h0,kkt2))
            for f4 in range(2):
                nc.scalar.dma_start(
                    out=xk_sb[:, 4 * f4:4 * f4 + 4, 512:1024],
                    in_=xkT_r[:, 4 * f4:4 * f4 + 4, 512:1024])
            # mask chunk 0 + V inputs early (V(0) pulled ~slot 4)
            nm_t = {}
            def _nmload0(kkg):
                t = nmpool.tile([128, 2, 1024], BF16, name=f"nm0{kkg}",
                                tag="nm")
                nm_t[(0, kkg)] = t
                nc.scalar.dma_start(
                    out=t[:], in_=notmT_r[:, kkg * 2:kkg * 2 + 2, 0:1024])
            _nmload0(0)
            xv_t = {}
            for kkt in range(2):
                xv_t[kkt] = xvpool.tile([128, NF, 128], BF16,
                                        name=f"xv{kkt}", tag="xv")
                nc.scalar.dma_start(
                    out=xv_t[kkt][:],
                    in_=xvT_r[:, :, kkt * 128:(kkt + 1) * 128])
            for f4 in range(2):
                nc.sync.dma_start(
                    out=wv_sb[:, 4 * f4:4 * f4 + 4, :],
                    in_=wvT_r[:, 4 * f4:4 * f4 + 4, :])
            _nmload0(1)
            _nmload0(2)
            _nmload0(3)
            # wq cols 256:512 (pairs 2,3 - needed from h4)
            for f4 in range(2):
                nc.sync.dma_start(out=wq_sb[:, 4 * f4:4 * f4 + 4, 256:512],
                                  in_=wqT_r[:, 4 * f4:4 * f4 + 4, 256:512])
            for kkg in range(4, 8):
                _nmload0(kkg)
            # xk blocks 2-3, wo (bulk, later deadlines)
            for blk in range(2, 4):
                for f4 in range(2):
                    nc.scalar.dma_start(
                        out=xk_sb[:, 4 * f4:4 * f4 + 4,
                                  blk * 512:(blk + 1) * 512],
                        in_=xkT_r[:, 4 * f4:4 * f4 + 4,
                                  blk * 512:(blk + 1) * 512])
            for f4 in range(2):
                nc.scalar.dma_start(
                    out=wo_sb[:, 2 * f4:2 * f4 + 2, :],
                    in_=woT_r[:, 2 * f4:2 * f4 + 2, :])

            # ---------------- thunks (each ~1.7us of PE) ----------------
            def exps(shape_view):
                ps = stpool.tile([128, 1024], F32, name="xps")
                return ps, ps[:, 0:shape_view].rearrange("p a -> p a")

            def kproj_thunk(p, blk):
                def run():
                    if blk == 0 and p + 1 < NP:
                        wk_t[p + 1] = wkpool.tile([128, NF, 128], BF16,
                                                  name=f"wk{p+1}", tag="wk")
                        nc.sync.dma_start(
                            out=wk_t[p + 1][:],
                            in_=wkT_r[:, :, (p + 1) * 128:(p + 2) * 128])
                    psv = xpspool.tile([128, 512], F32, name="kps",
                                        tag="xps")
                    for fi in range(NF):
                        nc.tensor.matmul(
                            psv, wk_t[p][:, fi, :],
                            xk_sb[:, fi, blk * 512:(blk + 1) * 512],
                            start=(fi == 0), stop=(fi == NF - 1))
                    nc.vector.tensor_scalar(
                        out=kt[:, p, blk * 512:(blk + 1) * 512],
                        in0=psv, scalar1=bk_sb[:, p:p + 1],
                        scalar2=None, op0=Alu.add)
                return run

            def qproj_thunk(ck, pgrp):
                def run():
                    if pgrp == 1 and ck + 4 < 8:
                        nck = ck + 4
                        xq_t[nck] = xqpool.tile([128, NF, 256], BF16,
                                                name=f"xq{nck}", tag="xq")
                        nc.sync.dma_start(
                            out=xq_t[nck][:],
                            in_=xqT_r[:, :, nck * 256:(nck + 1) * 256])
                    for p in (2 * pgrp, 2 * pgrp + 1):
                        ps = xpspool.tile([128, 512], F32, name="qps",
                                          tag="xps")
                        psv = ps[:, 0:256]
                        for fi in range(NF):
                            nc.tensor.matmul(
                                psv, wq_sb[:, fi, p * 128:(p + 1) * 128],
                                xq_t[ck][:, fi, :],
                                start=(fi == 0), stop=(fi == NF - 1))
                        nc.vector.tensor_scalar(
                            out=qt[:, p, ck * 256:(ck + 1) * 256],
                            in0=psv, scalar1=0.125,
                            scalar2=bq8_sb[:, p:p + 1],
                            op0=Alu.mult, op1=Alu.add)
                return run

            def vproj_thunk(kkt):
                def run():
                    if kkt + 2 < NKT:
                        xv_t[kkt + 2] = xvpool.tile([128, NF, 128], BF16,
                                                    name=f"xv{kkt+2}",
                                                    tag="xv")
                        nc.scalar.dma_start(
                            out=xv_t[kkt + 2][:],
                            in_=xvT_r[:, :, (kkt + 2) * 128:(kkt + 3) * 128])
                    psv = xpspool.tile([128, 512], F32, name="vps",
                                        tag="xps")
                    for fi in range(NF):
                        nc.tensor.matmul(
                            psv, xv_t[kkt][:, fi, :], wv_sb[:, fi, :],
                            start=(fi == 0), stop=(fi == NF - 1))
                    nc.vector.tensor_copy(
                        out=vaug[:, kkt, :, 0:64],
                        in_=psv.rearrange("p (h d) -> p h d", d=64))
                    emitted_v.add(kkt)
                return run

            def nmload_thunk(kkg):
                def run():
                    t = nmpool.tile([128, 2, 1024], BF16, name=f"nm1{kkg}",
                                    tag="nm")
                    nm_t[(1, kkg)] = t
                    for half in range(2):
                        nc.sync.dma_start(
                            out=t[:, half:half + 1, :],
                            in_=notmT_r[:, kkg * 2 + half:
                                        kkg * 2 + half + 1, 1024:2048])
                return run

            def outproj_thunk(qti, et, alt=False):
                qoff = qti * 1024
                def run():
                    for qh in range(2):
                        if alt and qh == 1:
                            pst = ovpool.tile([128, 4, 128], F32,
                                              name="ops2", tag="ov")
                            psv = pst.rearrange("p a b -> p (a b)")
                        elif alt and et % 2 == 1:
                            pst = stpool.tile([128, 1024], F32,
                                              name="ops3", tag="st")
                            psv = pst[:, 0:512]
                        else:
                            psv = xpspool.tile([128, 512], F32, name="ops",
                                               tag="xps")
                        for dc in range(NP):
                            nc.tensor.matmul(
                                psv, wo_sb[:, dc, et * 128:(et + 1) * 128],
                                ott[:, dc,
                                    qoff + qh * 512:qoff + (qh + 1) * 512],
                                start=(dc == 0), stop=(dc == NP - 1))
                        stg = ostg.tile([128, 512], F32, name="cstg")
                        nc.vector.tensor_scalar(
                            out=stg[:], in0=psv,
                            scalar1=bo2_sb[:, et:et + 1], scalar2=None,
                            op0=Alu.add)
                        nc.sync.dma_start(
                            out=outT[et * 128:(et + 1) * 128,
                                     qoff + qh * 512:qoff + (qh + 1) * 512],
                            in_=stg[:])
                return run

            emitted_v = set()

            def vproj_if_needed(kkt):
                if kkt not in emitted_v:
                    vproj_thunk(kkt)()

            # ---------------- upfront PE: minimum to start h0 ------------
            kproj_thunk(0, 0)()
            for ck in range(4):
                qproj_thunk(ck, 0)()

            # extras: {(qti,h): [(slot, thunk), ...]} (V is pulled on demand
            # by the PV drain, so only K/Q/nm/outproj are slotted here)
            X = {
                (0, 0): [(1, kproj_thunk(0, 1)), (5, kproj_thunk(0, 2)),
                         (9, kproj_thunk(0, 3))],
                (0, 1): [(0, kproj_thunk(1, 0)), (4, kproj_thunk(1, 1)),
                         (8, kproj_thunk(1, 2)), (12, kproj_thunk(1, 3))],
                (0, 2): [(2, kproj_thunk(2, 0)), (6, kproj_thunk(2, 1)),
                         (10, kproj_thunk(2, 2)), (14, kproj_thunk(2, 3))],
                (0, 3): [(1, qproj_thunk(0, 1)), (5, qproj_thunk(1, 1)),
                         (9, qproj_thunk(2, 1)), (13, qproj_thunk(3, 1))],
                (0, 4): [(1, kproj_thunk(3, 0)), (5, kproj_thunk(3, 1)),
                         (9, kproj_thunk(3, 2)), (13, kproj_thunk(3, 3))],
                (0, 5): [(1, qproj_thunk(4, 0)), (5, qproj_thunk(4, 1)),
                         (9, qproj_thunk(5, 0)), (13, qproj_thunk(5, 1)),
                         (11, nmload_thunk(0)), (15, nmload_thunk(1))],
                (0, 6): [(1, qproj_thunk(6, 0)), (5, qproj_thunk(6, 1)),
                         (3, nmload_thunk(2)), (7, nmload_thunk(3)),
                         (9, nmload_thunk(4)), (13, nmload_thunk(5))],
                (0, 7): [(1, qproj_thunk(7, 0)), (5, qproj_thunk(7, 1)),
                         (9, nmload_thunk(6)), (13, nmload_thunk(7))],
                (1, 0): [(2, outproj_thunk(0, 0)), (10, outproj_thunk(0, 1))],
                (1, 1): [(2, outproj_thunk(0, 2)), (10, outproj_thunk(0, 3))],
                (1, 2): [(2, outproj_thunk(0, 4)), (10, outproj_thunk(0, 5))],
                (1, 3): [(2, outproj_thunk(0, 6)), (10, outproj_thunk(0, 7))],
            }

            # ---------------- spine with pending-PV queue ----------------
            pvq = []   # entries: [qti, h, kkt, pt, ov, remaining-first-flag]
            seg_left = {}  # (qti,h) -> count of unemitted PVs

            def head_finish(qti, h, ov, otn):
                p, lo = h // 2, (h % 2) * 64
                qoff = qti * 1024
                onorm = onpool.tile([128, 8, 65], F32, name="onorm")
                nc.vector.tensor_copy(out=onorm[:, 0:4, :],
                                      in_=ov["A"][:, :, 0:65])
                nc.vector.tensor_copy(out=onorm[:, 4:8, :],
                                      in_=ov["B"][:, :, 0:65])
                for qs in range(8):
                    nc.gpsimd.normalize_recip(
                        otn[:, qs, lo:lo + 64],
                        onorm[:, qs, 0:64],
                        onorm[:, qs, 64:65])
                if h % 2 == 1:
                    for qs in range(8):
                        nc.sync.dma_start_transpose(
                            ott[:, p,
                                qoff + qs * 128:qoff + (qs + 1) * 128],
                            otn[:, qs, :])

            def emit_one_pv():
                qti, h, kkt, ptt, ov, otn = pvq.pop(0)
                if "A" not in ov:
                    ov["A"] = ovpool.tile([128, 4, 128], F32, name="ovA",
                                          tag="ov")
                    ov["B"] = ovpool.tile([128, 4, 128], F32, name="ovB",
                                          tag="ov")
                for qs in range(8):
                    tgt = ov["A"] if qs < 4 else ov["B"]
                    nc.tensor.matmul(
                        tgt[:, qs % 4, 0:65],
                        ptt[:, qs * 128:(qs + 1) * 128],
                        vaug[:, kkt, h, :],
                        start=(kkt == 0 and qs % 4 == 0),
                        stop=(kkt == NKT - 1 and qs % 4 == 3),
                        skip_group_check=qs % 4 not in (0, 3))
                seg_left[(qti, h)] -= 1
                if seg_left[(qti, h)] == 0:
                    head_finish(qti, h, ov, otn)

            def drain_pv(limit_pending):
                while len(pvq) > limit_pending:
                    vproj_if_needed(pvq[0][2])
                    emit_one_pv()

            otn = None
            for qti in range(2):
                qoff = qti * 1024
                for h in range(8):
                    p, lo = h // 2, (h % 2) * 64
                    slots = [[] for _ in range(16)]
                    for s, th in X.get((qti, h), []):
                        slots[s].append(th)
                    ov = {}
                    if h % 2 == 0:
                        otn = otnpool.tile([128, 8, 128], BF16,
                                           name="otn", tag="otn")
                    seg_left[(qti, h)] = NKT
                    lag = 0 if (qti, h) == (1, 7) else LAG
                    for kkt in range(NKT):
                        drain_pv(lag)
                        with tc.high_priority():
                            st = stpool.tile([128, 1024], F32, name="st",
                                             tag="st")
                            for j in range(2):
                                nc.tensor.matmul(
                                    st[:, j * 512:(j + 1) * 512],
                                    kt[lo:lo + 64, p,
                                       kkt * 128:(kkt + 1) * 128],
                                    qt[lo:lo + 64, p,
                                       qoff + j * 512:qoff + (j + 1) * 512],
                                    start=True, stop=True,
                                    tile_position=(lo, 0))
                            pt = ptpool.tile([128, 1024], BF16, name="pt")
                            nc.scalar.activation(pt[:], st[:], Act.Exp)
                            nmt = nm_t[(qti, kkt // 2)]
                            nc.vector.tensor_tensor(
                                out=pt[:], in0=pt[:],
                                in1=nmt[:, kkt % 2, :], op=Alu.mult)
                        pvq.append([qti, h, kkt, pt, ov, otn])
                        for th in slots[kkt]:
                            th()
                    drain_pv(0)
            # tail: q-tile-1 output projection (extra PSUM depth via the
            # now-idle ov pool keeps the PE fed and at full p-state)
            for et in range(NF):
                outproj_thunk(1, et, alt=True)()
    nc.compile()
    return nc


def _get_nc():
    if "nc" not in _STATE:
        _STATE["nc"] = build_nc()
    return _STATE["nc"]


def kernel(query, key, value, mask, Wq, bq, Wk, bk, Wv, bv, Wo, bo):
    query = np.asarray(query, dtype=np.float32)
    key = np.asarray(key, dtype=np.float32)
    value = np.asarray(value, dtype=np.float32)
    mask = np.asarray(mask)
    Wq = np.asarray(Wq, dtype=np.float32)
    Wk = np.asarray(Wk, dtype=np.float32)
    Wv = np.asarray(Wv, dtype=np.float32)
    Wo = np.asarray(Wo, dtype=np.float32)
    bq = np.asarray(bq, dtype=np.float32)
    bk = np.asarray(bk, dtype=np.float32)
    bv = np.asarray(bv, dtype=np.float32)
    bo = np.asarray(bo, dtype=np.float32)

    bf = ml_dtypes.bfloat16
    xq_b, xk_b, xv_b, nm_b = [], [], [], []
    for b in range(4):
        xq_b.append(np.ascontiguousarray(query[b].T).astype(bf))
        xk_b.append(np.ascontiguousarray(key[b].T).astype(bf))
        xv_b.append(np.ascontiguousarray(value[b].T).astype(bf))
        nm_b.append(np.ascontiguousarray(
            (~mask[b, 0]).T.astype(bf)))
    wq_g, wk_g, wv_g, wo_g, bq_g, bk_g, bo_g = [], [], [], [], [], [], []
    for hg in range(2):
        dsl = slice(hg * 512, (hg + 1) * 512)
        wq_g.append(np.ascontiguousarray(Wq.T[:, dsl]).astype(bf))
        wk_g.append(np.ascontiguousarray(Wk.T[:, dsl]).astype(bf))
        wv_g.append(np.ascontiguousarray(Wv.T[:, dsl]).astype(bf))
        wo_g.append(np.ascontiguousarray(Wo[:, dsl].T).astype(bf))
        bq_g.append(np.ascontiguousarray((bq[dsl] / 8.0).reshape(NP, 128).T))
        bk_g.append(np.ascontiguousarray(bk[dsl].reshape(NP, 128).T))
        bo_g.append(np.ascontiguousarray(
            (bo / 2.0 + Wo[:, dsl] @ bv[dsl]).reshape(NF, 128).T))

    in_maps = []
    for c in range(N_CORES):
        b, hg = c // 2, c % 2
        in_maps.append({
            "xqT": xq_b[b], "xkT": xk_b[b], "xvT": xv_b[b],
            "wqT": wq_g[hg], "wkT": wk_g[hg], "wvT": wv_g[hg],
            "woT": wo_g[hg],
            "bq8": bq_g[hg], "bk_l": bk_g[hg], "bo2": bo_g[hg],
            "notmT": nm_b[b],
        })

    nc = _get_nc()
    res = run_bass_kernel_spmd(nc, in_maps, core_ids=list(range(N_CORES)))
    out = np.empty((4, SQ, EMBED), dtype=np.float32)
    for b in range(4):
        acc = res.results[2 * b]["outT"] + res.results[2 * b + 1]["outT"]
        out[b] = acc.T
    return out


# revision 9
# speedup vs baseline: 1.2442x; 1.0016x over previous
"""MultiHeadAttention TRN2 kernel, 8-core SPMD — interleaved emission (V3.1).

Sharding: core c -> batch b=c//2, head-group hg=c%2 (8 heads / 512 dims).
All inputs bf16 except biases. Spine is q-tile-major attention; K/Q/V
projections and the q-tile-0 output projection are "extras" interleaved
into the PE stream so the ACT engine (exp, the ~266us floor) stays fed.
PV matmuls trail the ST/exp stream through a pending queue (P stationary,
free=65 -> half PE cost) and may slip across segment boundaries.

PSUM accumulation groups are per 2KB zero region (bank): start zeroes the
whole bank -> one start/stop per bank, interior matmuls skip group check.
Extras borrow PSUM from the st pool (ov pool slots live too long).
"""
import numpy as np
import ml_dtypes

import concourse.bass as bass
import concourse.mybir as mybir
import concourse.tile as tile
from concourse import bacc
from concourse.bass_utils import run_bass_kernel_spmd

F32 = mybir.dt.float32
BF16 = mybir.dt.bfloat16
Act = mybir.ActivationFunctionType
Alu = mybir.AluOpType

EMBED = 1024
SK = 2048
SQ = 2048
NF = 8
NKT = 16
NP = 4
N_CORES = 8
LAG = 3

_STATE = {}


def build_nc():
    nc = bacc.Bacc("TRN2", target_bir_lowering=False)
    xqT = nc.dram_tensor("xqT", [EMBED, SQ], BF16, kind="ExternalInput")
    xkT = nc.dram_tensor("xkT", [EMBED, SK], BF16, kind="ExternalInput")
    xvT = nc.dram_tensor("xvT", [EMBED, SK], BF16, kind="ExternalInput")
    wqT = nc.dram_tensor("wqT", [EMBED, 512], BF16, kind="ExternalInput")
    wkT = nc.dram_tensor("wkT", [EMBED, 512], BF16, kind="ExternalInput")
    wvT = nc.dram_tensor("wvT", [EMBED, 512], BF16, kind="ExternalInput")
    woT = nc.dram_tensor("woT", [512, EMBED], BF16, kind="ExternalInput")
    bq8 = nc.dram_tensor("bq8", [128, NP], F32, kind="ExternalInput")
    bk_l = nc.dram_tensor("bk_l", [128, NP], F32, kind="ExternalInput")
    bo2 = nc.dram_tensor("bo2", [128, NF], F32, kind="ExternalInput")
    notmT = nc.dram_tensor("notmT", [SK, SQ], BF16, kind="ExternalInput")
    outT = nc.dram_tensor("outT", [EMBED, SQ], F32, kind="ExternalOutput")

    xqT_r = xqT.rearrange("(t p) q -> p t q", p=128)
    xkT_r = xkT.rearrange("(t p) k -> p t k", p=128)
    xvT_r = xvT.rearrange("(t p) k -> p t k", p=128)
    wqT_r = wqT.rearrange("(t p) n -> p t n", p=128)
    wkT_r = wkT.rearrange("(t p) n -> p t n", p=128)
    wvT_r = wvT.rearrange("(t p) n -> p t n", p=128)
    woT_r = woT.rearrange("(t p) n -> p t n", p=128)
    notmT_r = notmT.rearrange("(t p) q -> p t q", p=128)

    with tile.TileContext(nc) as tc:
        with tc.tile_pool(name="persist", bufs=1) as pp, \
             tc.tile_pool(name="bias", bufs=1) as bp, \
             tc.tile_pool(name="nmch", bufs=8) as nmpool, \
             tc.tile_pool(name="wkch", bufs=2) as wkpool, \
             tc.tile_pool(name="xqch", bufs=4) as xqpool, \
             tc.tile_pool(name="xvch", bufs=2) as xvpool, \
             tc.tile_pool(name="apt", bufs=6) as ptpool, \
             tc.tile_pool(name="aon", bufs=1) as onpool, \
             tc.tile_pool(name="aotn", bufs=2) as otnpool, \
             tc.tile_pool(name="aost", bufs=4) as ostg, \
             tc.tile_pool(name="ast", bufs=2, space="PSUM") as stpool, \
             tc.tile_pool(name="axps", bufs=1, space="PSUM") as xpspool, \
             tc.tile_pool(name="aov", bufs=3, space="PSUM") as ovpool:
            kt = pp.tile([128, NP, SK], BF16, name="kt")
            qt = pp.tile([128, NP, SQ], BF16, name="qt")
            vaug = pp.tile([128, NKT, 8, 65], BF16, name="vaug")
            xk_sb = pp.tile([128, NF, SK], BF16, name="xk_sb")
            wq_sb = pp.tile([128, NF, 512], BF16, name="wq_sb")
            wv_sb = pp.tile([128, NF, 512], BF16, name="wv_sb")
            wo_sb = pp.tile([128, NP, EMBED], BF16, name="wo_sb")
            ott = pp.tile([128, NP, SQ], BF16, name="ott")
            bq8_sb = bp.tile([128, NP], F32, name="bq8_sb")
            bk_sb = bp.tile([128, NP], F32, name="bk_sb")
            bo2_sb = bp.tile([128, NF], F32, name="bo2_sb")
            nc.vector.memset(vaug[:, :, :, 64:65], 1.0)

            # ---------------- DMA preamble (urgency-ordered) ----------
            nc.sync.dma_start(out=bk_sb[:], in_=bk_l[:, :])
            nc.sync.dma_start(out=bq8_sb[:], in_=bq8[:, :])
            nc.sync.dma_start(out=bo2_sb[:], in_=bo2[:, :])
            wk_t = {}
            wk_t[0] = wkpool.tile([128, NF, 128], BF16, name="wk0", tag="wk")
            for f4 in range(2):
                nc.sync.dma_start(out=wk_t[0][:, 4 * f4:4 * f4 + 4, :],
                                  in_=wkT_r[:, 4 * f4:4 * f4 + 4, 0:128])
            # xk block 0 fine-grained (feeds K(p0,b0) asap)
            for f2 in range(4):
                nc.scalar.dma_start(out=xk_sb[:, 2 * f2:2 * f2 + 2, 0:512],
                                    in_=xkT_r[:, 2 * f2:2 * f2 + 2, 0:512])
            # wq cols 0:256 (pairs 0,1) first
            for f4 in range(2):
                nc.sync.dma_start(out=wq_sb[:, 4 * f4:4 * f4 + 4, 0:256],
                                  in_=wqT_r[:, 4 * f4:4 * f4 + 4, 0:256])
            xq_t = {}
            for ck in range(4):
                xq_t[ck] = xqpool.tile([128, NF, 256], BF16,
                                       name=f"xq{ck}", tag="xq")
                nc.sync.dma_start(
                    out=xq_t[ck][:],
                    in_=xqT_r[:, :, ck * 256:(ck + 1) * 256])
            # xk block 1 immediately (K(p0,b1) gates ST(h0,kkt2))
            for f4 in range(2):
                nc.scalar.dma_start(
                    out=xk_sb[:, 4 * f4:4 * f4 + 4, 512:1024],
                    in_=xkT_r[:, 4 * f4:4 * f4 + 4, 512:1024])
            # mask chunk 0 + V inputs early (V(0) pulled ~slot 4)
            nm_t = {}
            def _nmload0(kkg):
                t = nmpool.tile([128, 2, 1024], BF16, name=f"nm0{kkg}",
                                tag="nm")
                nm_t[(0, kkg)] = t
                nc.scalar.dma_start(
                    out=t[:], in_=notmT_r[:, kkg * 2:kkg * 2 + 2, 0:1024])
            _nmload0(0)
            xv_t = {}
            for kkt in range(2):
                xv_t[kkt] = xvpool.tile([128, NF, 128], BF16,
                                        name=f"xv{kkt}", tag="xv")
                nc.scalar.dma_start(
                    out=xv_t[kkt][:],
                    in_=xvT_r[:, :, kkt * 128:(kkt + 1) * 128])
            for f4 in range(2):
                nc.sync.dma_start(
                    out=wv_sb[:, 4 * f4:4 * f4 + 4, :],
                    in_=wvT_r[:, 4 * f4:4 * f4 + 4, :])
            _nmload0(1)
            _nmload0(2)
            _nmload0(3)
            # wq cols 256:512 (pairs 2,3 - needed from h4)
            for f4 in range(2):
                nc.sync.dma_start(out=wq_sb[:, 4 * f4:4 * f4 + 4, 256:512],
                                  in_=wqT_r[:, 4 * f4:4 * f4 + 4, 256:512])
            for kkg in range(4, 8):
                _nmload0(kkg)
            # xk blocks 2-3, wo (bulk, later deadlines)
            for blk in range(2, 4):
                for f4 in range(2):
                    nc.scalar.dma_start(
                        out=xk_sb[:, 4 * f4:4 * f4 + 4,
                                  blk * 512:(blk + 1) * 512],
                        in_=xkT_r[:, 4 * f4:4 * f4 + 4,
                                  blk * 512:(blk + 1) * 512])
            for f4 in range(2):
                nc.scalar.dma_start(
                    out=wo_sb[:, 2 * f4:2 * f4 + 2, :],
                    in_=woT_r[:, 2 * f4:2 * f4 + 2, :])

            # ---------------- thunks (each ~1.7us of PE) ----------------
            def exps(shape_view):
                ps = stpool.tile([128, 1024], F32, name="xps")
                return ps, ps[:, 0:shape_view].rearrange("p a -> p a")

            def kproj_thunk(p, blk):
                def run():
                    if blk == 0 and p + 1 < NP:
                        wk_t[p + 1] = wkpool.tile([128, NF, 128], BF16,
                                                  name=f"wk{p+1}", tag="wk")
                        nc.sync.dma_start(
                            out=wk_t[p + 1][:],
                            in_=wkT_r[:, :, (p + 1) * 128:(p + 2) * 128])
                    psv = xpspool.tile([128, 512], F32, name="kps",
                                        tag="xps")
                    for fi in range(NF):
                        nc.tensor.matmul(
                            psv, wk_t[p][:, fi, :],
                            xk_sb[:, fi, blk * 512:(blk + 1) * 512],
                            start=(fi == 0), stop=(fi == NF - 1))
                    nc.vector.tensor_scalar(
                        out=kt[:, p, blk * 512:(blk + 1) * 512],
                        in0=psv, scalar1=bk_sb[:, p:p + 1],
                        scalar2=None, op0=Alu.add)
                return run

            def qproj_thunk(ck, pgrp):
                def run():
                    if pgrp == 1 and ck + 4 < 8:
                        nck = ck + 4
                        xq_t[nck] = xqpool.tile([128, NF, 256], BF16,
                                                name=f"xq{nck}", tag="xq")
                        nc.sync.dma_start(
                            out=xq_t[nck][:],
                            in_=xqT_r[:, :, nck * 256:(nck + 1) * 256])
                    for p in (2 * pgrp, 2 * pgrp + 1):
                        ps = xpspool.tile([128, 512], F32, name="qps",
                                          tag="xps")
                        psv = ps[:, 0:256]
                        for fi in range(NF):
                            nc.tensor.matmul(
                                psv, wq_sb[:, fi, p * 128:(p + 1) * 128],
                                xq_t[ck][:, fi, :],
                                start=(fi == 0), stop=(fi == NF - 1))
                        nc.vector.tensor_scalar(
                            out=qt[:, p, ck * 256:(ck + 1) * 256],
                            in0=psv, scalar1=0.125,
                            scalar2=bq8_sb[:, p:p + 1],
                            op0=Alu.mult, op1=Alu.add)
                return run

            def vproj_thunk(kkt):
                def run():
                    if kkt + 2 < NKT:
                        xv_t[kkt + 2] = xvpool.tile([128, NF, 128], BF16,
                                                    name=f"xv{kkt+2}",
                                                    tag="xv")
                        nc.scalar.dma_start(
                            out=xv_t[kkt + 2][:],
                            in_=xvT_r[:, :, (kkt + 2) * 128:(kkt + 3) * 128])
                    psv = xpspool.tile([128, 512], F32, name="vps",
                                        tag="xps")
                    for fi in range(NF):
                        nc.tensor.matmul(
                            psv, xv_t[kkt][:, fi, :], wv_sb[:, fi, :],
                            start=(fi == 0), stop=(fi == NF - 1))
                    nc.vector.tensor_copy(
                        out=vaug[:, kkt, :, 0:64],
                        in_=psv.rearrange("p (h d) -> p h d", d=64))
                    emitted_v.add(kkt)
                return run

            def nmload_thunk(kkg):
                def run():
                    t = nmpool.tile([128, 2, 1024], BF16, name=f"nm1{kkg}",
                                    tag="nm")
                    nm_t[(1, kkg)] = t
                    for half in range(2):
                        nc.sync.dma_start(
                            out=t[:, half:half + 1, :],
                            in_=notmT_r[:, kkg * 2 + half:
                                        kkg * 2 + half + 1, 1024:2048])
                return run

            def outproj_thunk(qti, et, alt=False):
                qoff = qti * 1024
                def run():
                    for qh in range(2):
                        if alt and qh == 1:
                            pst = ovpool.tile([128, 4, 128], F32,
                                              name="ops2", tag="ov")
                            psv = pst.rearrange("p a b -> p (a b)")
                        elif alt and et % 2 == 1:
                            pst = stpool.tile([128, 1024], F32,
                                              name="ops3", tag="st")
                            psv = pst[:, 0:512]
                        else:
                            psv = xpspool.tile([128, 512], F32, name="ops",
                                               tag="xps")
                        for dc in range(NP):
                            nc.tensor.matmul(
                                psv, wo_sb[:, dc, et * 128:(et + 1) * 128],
                                ott[:, dc,
                                    qoff + qh * 512:qoff + (qh + 1) * 512],
                                start=(dc == 0), stop=(dc == NP - 1))
                        stg = ostg.tile([128, 512], F32, name="cstg")
                        nc.vector.tensor_scalar(
                            out=stg[:], in0=psv,
                            scalar1=bo2_sb[:, et:et + 1], scalar2=None,
                            op0=Alu.add)
                        nc.sync.dma_start(
                            out=outT[et * 128:(et + 1) * 128,
                                     qoff + qh * 512:qoff + (qh + 1) * 512],
                            in_=stg[:])
                return run

            emitted_v = set()

            def vproj_if_needed(kkt):
                if kkt not in emitted_v:
                    vproj_thunk(kkt)()

            # ---------------- upfront PE: minimum to start h0 ------------
            kproj_thunk(0, 0)()
            for ck in range(4):
                qproj_thunk(ck, 0)()

            # extras: {(qti,h): [(slot, thunk), ...]} (V is pulled on demand
            # by the PV drain, so only K/Q/nm/outproj are slotted here)
            X = {
                (0, 0): [(1, kproj_thunk(0, 1)), (5, kproj_thunk(0, 2)),
                         (9, kproj_thunk(0, 3))],
                (0, 1): [(0, kproj_thunk(1, 0)), (4, kproj_thunk(1, 1)),
                         (8, kproj_thunk(1, 2)), (12, kproj_thunk(1, 3))],
                (0, 2): [(2, kproj_thunk(2, 0)), (6, kproj_thunk(2, 1)),
                         (10, kproj_thunk(2, 2)), (14, kproj_thunk(2, 3))],
                (0, 3): [(1, qproj_thunk(0, 1)), (5, qproj_thunk(1, 1)),
                         (9, qproj_thunk(2, 1)), (13, qproj_thunk(3, 1))],
                (0, 4): [(1, kproj_thunk(3, 0)), (5, kproj_thunk(3, 1)),
                         (9, kproj_thunk(3, 2)), (13, kproj_thunk(3, 3))],
                (0, 5): [(1, qproj_thunk(4, 0)), (5, qproj_thunk(4, 1)),
                         (9, qproj_thunk(5, 0)), (13, qproj_thunk(5, 1)),
                         (11, nmload_thunk(0)), (15, nmload_thunk(1))],
                (0, 6): [(1, qproj_thunk(6, 0)), (5, qproj_thunk(6, 1)),
                         (3, nmload_thunk(2)), (7, nmload_thunk(3)),
                         (9, nmload_thunk(4)), (13, nmload_thunk(5))],
                (0, 7): [(1, qproj_thunk(7, 0)), (5, qproj_thunk(7, 1)),
                         (9, nmload_thunk(6)), (13, nmload_thunk(7))],
                (1, 0): [(2, outproj_thunk(0, 0)), (10, outproj_thunk(0, 1))],
                (1, 1): [(2, outproj_thunk(0, 2)), (10, outproj_thunk(0, 3))],
                (1, 2): [(2, outproj_thunk(0, 4)), (10, outproj_thunk(0, 5))],
                (1, 3): [(2, outproj_thunk(0, 6)), (10, outproj_thunk(0, 7))],
            }

            # ---------------- spine with pending-PV queue ----------------
            pvq = []   # entries: [qti, h, kkt, pt, ov, remaining-first-flag]
            seg_left = {}  # (qti,h) -> count of unemitted PVs

            def head_finish(qti, h, ov, otn):
                p, lo = h // 2, (h % 2) * 64
                qoff = qti * 1024
                onorm = onpool.tile([128, 8, 65], F32, name="onorm")
                nc.vector.tensor_copy(out=onorm[:, 0:4, :],
                                      in_=ov["A"][:, :, 0:65])
                nc.vector.tensor_copy(out=onorm[:, 4:8, :],
                                      in_=ov["B"][:, :, 0:65])
                for qs in range(8):
                    nc.gpsimd.normalize_recip(
                        otn[:, qs, lo:lo + 64],
                        onorm[:, qs, 0:64],
                        onorm[:, qs, 64:65])
                if h % 2 == 1:
                    for qs in range(8):
                        nc.sync.dma_start_transpose(
                            ott[:, p,
                                qoff + qs * 128:qoff + (qs + 1) * 128],
                            otn[:, qs, :])

            def emit_one_pv():
                qti, h, kkt, ptt, ov, otn = pvq.pop(0)
                if "A" not in ov:
                    ov["A"] = ovpool.tile([128, 4, 128], F32, name="ovA",
                                          tag="ov")
                    ov["B"] = ovpool.tile([128, 4, 128], F32, name="ovB",
                                          tag="ov")
                for qs in range(8):
                    tgt = ov["A"] if qs < 4 else ov["B"]
                    nc.tensor.matmul(
                        tgt[:, qs % 4, 0:65],
                        ptt[:, qs * 128:(qs + 1) * 128],
                        vaug[:, kkt, h, :],
                        start=(kkt == 0 and qs % 4 == 0),
                        stop=(kkt == NKT - 1 and qs % 4 == 3),
                        skip_group_check=qs % 4 not in (0, 3))
                seg_left[(qti, h)] -= 1
                if seg_left[(qti, h)] == 0:
                    head_finish(qti, h, ov, otn)

            def drain_pv(limit_pending):
                while len(pvq) > limit_pending:
                    vproj_if_needed(pvq[0][2])
                    emit_one_pv()

            otn = None
            for qti in range(2):
                qoff = qti * 1024
                for h in range(8):
                    p, lo = h // 2, (h % 2) * 64
                    slots = [[] for _ in range(16)]
                    for s, th in X.get((qti, h), []):
                        slots[s].append(th)
                    ov = {}
                    if h % 2 == 0:
                        otn = otnpool.tile([128, 8, 128], BF16,
                                           name="otn", tag="otn")
                    seg_left[(qti, h)] = NKT
                    lag = 0 if (qti, h) == (1, 7) else LAG
                    for kkt in range(NKT):
                        drain_pv(lag)
                        with tc.high_priority():
                            st = stpool.tile([128, 1024], F32, name="st",
                                             tag="st")
                            for j in range(2):
                                nc.tensor.matmul(
                                    st[:, j * 512:(j + 1) * 512],
                                    kt[lo:lo + 64, p,
                                       kkt * 128:(kkt + 1) * 128],
                                    qt[lo:lo + 64, p,
                                       qoff + j * 512:qoff + (j + 1) * 512],
                                    start=True, stop=True,
                                    tile_position=(lo, 0))
                            pt = ptpool.tile([128, 1024], BF16, name="pt")
                            nc.scalar.activation(pt[:], st[:], Act.Exp)
                            nmt = nm_t[(qti, kkt // 2)]
                            nc.vector.tensor_tensor(
                                out=pt[:], in0=pt[:],
                                in1=nmt[:, kkt % 2, :], op=Alu.mult)
                        pvq.append([qti, h, kkt, pt, ov, otn])
                        for th in slots[kkt]:
                            th()
                    drain_pv(0)
            # tail: q-tile-1 output projection (extra PSUM depth via the
            # now-idle ov pool keeps the PE fed and at full p-state)
            for et in range(NF):
                outproj_thunk(1, et, alt=True)()
    nc.compile()
    return nc


def _get_nc():
    if "nc" not in _STATE:
        _STATE["nc"] = build_nc()
    return _STATE["nc"]


def kernel(query, key, value, mask, Wq, bq, Wk, bk, Wv, bv, Wo, bo):
    query = np.asarray(query, dtype=np.float32)
    key = np.asarray(key, dtype=np.float32)
    value = np.asarray(value, dtype=np.float32)
    mask = np.asarray(mask)
    Wq = np.asarray(Wq, dtype=np.float32)
    Wk = np.asarray(Wk, dtype=np.float32)
    Wv = np.asarray(Wv, dtype=np.float32)
    Wo = np.asarray(Wo, dtype=np.float32)
    bq = np.asarray(bq, dtype=np.float32)
    bk = np.asarray(bk, dtype=np.float32)
    bv = np.asarray(bv, dtype=np.float32)
    bo = np.asarray(bo, dtype=np.float32)

    bf = ml_dtypes.bfloat16
    xq_b, xk_b, xv_b, nm_b = [], [], [], []
    for b in range(4):
        xq_b.append(np.ascontiguousarray(query[b].T).astype(bf))
        xk_b.append(np.ascontiguousarray(key[b].T).astype(bf))
        xv_b.append(np.ascontiguousarray(value[b].T).astype(bf))
        nm_b.append(np.ascontiguousarray(
            (~mask[b, 0]).T.astype(bf)))
    wq_g, wk_g, wv_g, wo_g, bq_g, bk_g, bo_g = [], [], [], [], [], [], []
    for hg in range(2):
        dsl = slice(hg * 512, (hg + 1) * 512)
        wq_g.append(np.ascontiguousarray(Wq.T[:, dsl]).astype(bf))
        wk_g.append(np.ascontiguousarray(Wk.T[:, dsl]).astype(bf))
        wv_g.append(np.ascontiguousarray(Wv.T[:, dsl]).astype(bf))
        wo_g.append(np.ascontiguousarray(Wo[:, dsl].T).astype(bf))
        bq_g.append(np.ascontiguousarray((bq[dsl] / 8.0).reshape(NP, 128).T))
        bk_g.append(np.ascontiguousarray(bk[dsl].reshape(NP, 128).T))
        bo_g.append(np.ascontiguousarray(
            (bo / 2.0 + Wo[:, dsl] @ bv[dsl]).reshape(NF, 128).T))

    in_maps = []
    for c in range(N_CORES):
        b, hg = c // 2, c % 2
        in_maps.append({
            "xqT": xq_b[b], "xkT": xk_b[b], "xvT": xv_b[b],
            "wqT": wq_g[hg], "wkT": wk_g[hg], "wvT": wv_g[hg],
            "woT": wo_g[hg],
            "bq8": bq_g[hg], "bk_l": bk_g[hg], "bo2": bo_g[hg],
            "notmT": nm_b[b],
        })

    nc = _get_nc()
    res = run_bass_kernel_spmd(nc, in_maps, core_ids=list(range(N_CORES)))
    out = np.empty((4, SQ, EMBED), dtype=np.float32)
    for b in range(4):
        acc = res.results[2 * b]["outT"] + res.results[2 * b + 1]["outT"]
        out[b] = acc.T
    return out


# revision 10
# speedup vs baseline: 1.2492x; 1.0040x over previous
"""MultiHeadAttention TRN2 kernel, 8-core SPMD — interleaved emission (V3.1).

Sharding: core c -> batch b=c//2, head-group hg=c%2 (8 heads / 512 dims).
All inputs bf16 except biases. Spine is q-tile-major attention; K/Q/V
projections and the q-tile-0 output projection are "extras" interleaved
into the PE stream so the ACT engine (exp, the ~266us floor) stays fed.
PV matmuls trail the ST/exp stream through a pending queue (P stationary,
free=65 -> half PE cost) and may slip across segment boundaries.

PSUM accumulation groups are per 2KB zero region (bank): start zeroes the
whole bank -> one start/stop per bank, interior matmuls skip group check.
Extras borrow PSUM from the st pool (ov pool slots live too long).
"""
import numpy as np
import ml_dtypes

import concourse.bass as bass
import concourse.mybir as mybir
import concourse.tile as tile
from concourse import bacc
from concourse.bass_utils import run_bass_kernel_spmd

F32 = mybir.dt.float32
BF16 = mybir.dt.bfloat16
Act = mybir.ActivationFunctionType
Alu = mybir.AluOpType

EMBED = 1024
SK = 2048
SQ = 2048
NF = 8
NKT = 16
NP = 4
N_CORES = 8
LAG = 3

_STATE = {}


def build_nc():
    nc = bacc.Bacc("TRN2", target_bir_lowering=False)
    xqT = nc.dram_tensor("xqT", [EMBED, SQ], BF16, kind="ExternalInput")
    xkT = nc.dram_tensor("xkT", [EMBED, SK], BF16, kind="ExternalInput")
    xvT = nc.dram_tensor("xvT", [EMBED, SK], BF16, kind="ExternalInput")
    wqT = nc.dram_tensor("wqT", [EMBED, 512], BF16, kind="ExternalInput")
    wkT = nc.dram_tensor("wkT", [EMBED, 512], BF16, kind="ExternalInput")
    wvT = nc.dram_tensor("wvT", [EMBED, 512], BF16, kind="ExternalInput")
    woT = nc.dram_tensor("woT", [512, EMBED], BF16, kind="ExternalInput")
    bq8 = nc.dram_tensor("bq8", [128, NP], F32, kind="ExternalInput")
    bk_l = nc.dram_tensor("bk_l", [128, NP], F32, kind="ExternalInput")
    bo2 = nc.dram_tensor("bo2", [128, NF], F32, kind="ExternalInput")
    notmT = nc.dram_tensor("notmT", [SK, SQ], BF16, kind="ExternalInput")
    outT = nc.dram_tensor("outT", [EMBED, SQ], F32, kind="ExternalOutput")

    xqT_r = xqT.rearrange("(t p) q -> p t q", p=128)
    xkT_r = xkT.rearrange("(t p) k -> p t k", p=128)
    xvT_r = xvT.rearrange("(t p) k -> p t k", p=128)
    wqT_r = wqT.rearrange("(t p) n -> p t n", p=128)
    wkT_r = wkT.rearrange("(t p) n -> p t n", p=128)
    wvT_r = wvT.rearrange("(t p) n -> p t n", p=128)
    woT_r = woT.rearrange("(t p) n -> p t n", p=128)
    notmT_r = notmT.rearrange("(t p) q -> p t q", p=128)

    with tile.TileContext(nc) as tc:
        with tc.tile_pool(name="persist", bufs=1) as pp, \
             tc.tile_pool(name="bias", bufs=1) as bp, \
             tc.tile_pool(name="nmch", bufs=8) as nmpool, \
             tc.tile_pool(name="wkch", bufs=2) as wkpool, \
             tc.tile_pool(name="xqch", bufs=4) as xqpool, \
             tc.tile_pool(name="xvch", bufs=2) as xvpool, \
             tc.tile_pool(name="apt", bufs=6) as ptpool, \
             tc.tile_pool(name="aon", bufs=1) as onpool, \
             tc.tile_pool(name="aotn", bufs=2) as otnpool, \
             tc.tile_pool(name="aost", bufs=4) as ostg, \
             tc.tile_pool(name="ast", bufs=2, space="PSUM") as stpool, \
             tc.tile_pool(name="axps", bufs=2, space="PSUM") as xpspool, \
             tc.tile_pool(name="aov", bufs=2, space="PSUM") as ovpool:
            kt = pp.tile([128, NP, SK], BF16, name="kt")
            qt = pp.tile([128, NP, SQ], BF16, name="qt")
            vaug = pp.tile([128, NKT, 8, 65], BF16, name="vaug")
            xk_sb = pp.tile([128, NF, SK], BF16, name="xk_sb")
            wq_sb = pp.tile([128, NF, 512], BF16, name="wq_sb")
            wv_sb = pp.tile([128, NF, 512], BF16, name="wv_sb")
            wo_sb = pp.tile([128, NP, EMBED], BF16, name="wo_sb")
            ott = pp.tile([128, NP, SQ], BF16, name="ott")
            bq8_sb = bp.tile([128, NP], F32, name="bq8_sb")
            bk_sb = bp.tile([128, NP], F32, name="bk_sb")
            bo2_sb = bp.tile([128, NF], F32, name="bo2_sb")
            nc.vector.memset(vaug[:, :, :, 64:65], 1.0)

            # ---------------- DMA preamble (urgency-ordered) ----------
            nc.sync.dma_start(out=bk_sb[:], in_=bk_l[:, :])
            nc.sync.dma_start(out=bq8_sb[:], in_=bq8[:, :])
            nc.sync.dma_start(out=bo2_sb[:], in_=bo2[:, :])
            wk_t = {}
            wk_t[0] = wkpool.tile([128, NF, 128], BF16, name="wk0", tag="wk")
            for f4 in range(2):
                nc.sync.dma_start(out=wk_t[0][:, 4 * f4:4 * f4 + 4, :],
                                  in_=wkT_r[:, 4 * f4:4 * f4 + 4, 0:128])
            # xk block 0 fine-grained (feeds K(p0,b0) asap)
            for f2 in range(4):
                nc.scalar.dma_start(out=xk_sb[:, 2 * f2:2 * f2 + 2, 0:512],
                                    in_=xkT_r[:, 2 * f2:2 * f2 + 2, 0:512])
            # wq cols 0:256 (pairs 0,1) first
            for f4 in range(2):
                nc.sync.dma_start(out=wq_sb[:, 4 * f4:4 * f4 + 4, 0:256],
                                  in_=wqT_r[:, 4 * f4:4 * f4 + 4, 0:256])
            xq_t = {}
            for ck in range(4):
                xq_t[ck] = xqpool.tile([128, NF, 256], BF16,
                                       name=f"xq{ck}", tag="xq")
                nc.sync.dma_start(
                    out=xq_t[ck][:],
                    in_=xqT_r[:, :, ck * 256:(ck + 1) * 256])
            # xk block 1 immediately (K(p0,b1) gates ST(h0,kkt2))
            for f4 in range(2):
                nc.scalar.dma_start(
                    out=xk_sb[:, 4 * f4:4 * f4 + 4, 512:1024],
                    in_=xkT_r[:, 4 * f4:4 * f4 + 4, 512:1024])
            # mask chunk 0 + V inputs early (V(0) pulled ~slot 4)
            nm_t = {}
            def _nmload0(kkg):
                t = nmpool.tile([128, 2, 1024], BF16, name=f"nm0{kkg}",
                                tag="nm")
                nm_t[(0, kkg)] = t
                nc.scalar.dma_start(
                    out=t[:], in_=notmT_r[:, kkg * 2:kkg * 2 + 2, 0:1024])
            _nmload0(0)
            xv_t = {}
            for kkt in range(2):
                xv_t[kkt] = xvpool.tile([128, NF, 128], BF16,
                                        name=f"xv{kkt}", tag="xv")
                nc.scalar.dma_start(
                    out=xv_t[kkt][:],
                    in_=xvT_r[:, :, kkt * 128:(kkt + 1) * 128])
            for f4 in range(2):
                nc.sync.dma_start(
                    out=wv_sb[:, 4 * f4:4 * f4 + 4, :],
                    in_=wvT_r[:, 4 * f4:4 * f4 + 4, :])
            _nmload0(1)
            _nmload0(2)
            _nmload0(3)
            # wq cols 256:512 (pairs 2,3 - needed from h4)
            for f4 in range(2):
                nc.sync.dma_start(out=wq_sb[:, 4 * f4:4 * f4 + 4, 256:512],
                                  in_=wqT_r[:, 4 * f4:4 * f4 + 4, 256:512])
            for kkg in range(4, 8):
                _nmload0(kkg)
            # xk blocks 2-3, wo (bulk, later deadlines)
            for blk in range(2, 4):
                for f4 in range(2):
                    nc.scalar.dma_start(
                        out=xk_sb[:, 4 * f4:4 * f4 + 4,
                                  blk * 512:(blk + 1) * 512],
                        in_=xkT_r[:, 4 * f4:4 * f4 + 4,
                                  blk * 512:(blk + 1) * 512])
            for f4 in range(2):
                nc.scalar.dma_start(
                    out=wo_sb[:, 2 * f4:2 * f4 + 2, :],
                    in_=woT_r[:, 2 * f4:2 * f4 + 2, :])

            # ---------------- thunks (each ~1.7us of PE) ----------------
            def exps(shape_view):
                ps = stpool.tile([128, 1024], F32, name="xps")
                return ps, ps[:, 0:shape_view].rearrange("p a -> p a")

            def kproj_thunk(p, blk):
                def run():
                    if blk == 0 and p + 1 < NP:
                        wk_t[p + 1] = wkpool.tile([128, NF, 128], BF16,
                                                  name=f"wk{p+1}", tag="wk")
                        nc.sync.dma_start(
                            out=wk_t[p + 1][:],
                            in_=wkT_r[:, :, (p + 1) * 128:(p + 2) * 128])
                    psv = xpspool.tile([128, 512], F32, name="kps",
                                        tag="xps")
                    for fi in range(NF):
                        nc.tensor.matmul(
                            psv, wk_t[p][:, fi, :],
                            xk_sb[:, fi, blk * 512:(blk + 1) * 512],
                            start=(fi == 0), stop=(fi == NF - 1))
                    nc.vector.tensor_scalar(
                        out=kt[:, p, blk * 512:(blk + 1) * 512],
                        in0=psv, scalar1=bk_sb[:, p:p + 1],
                        scalar2=None, op0=Alu.add)
                return run

            def qproj_thunk(ck, pgrp):
                def run():
                    if pgrp == 1 and ck + 4 < 8:
                        nck = ck + 4
                        xq_t[nck] = xqpool.tile([128, NF, 256], BF16,
                                                name=f"xq{nck}", tag="xq")
                        nc.sync.dma_start(
                            out=xq_t[nck][:],
                            in_=xqT_r[:, :, nck * 256:(nck + 1) * 256])
                    for p in (2 * pgrp, 2 * pgrp + 1):
                        ps = xpspool.tile([128, 512], F32, name="qps",
                                          tag="xps")
                        psv = ps[:, 0:256]
                        for fi in range(NF):
                            nc.tensor.matmul(
                                psv, wq_sb[:, fi, p * 128:(p + 1) * 128],
                                xq_t[ck][:, fi, :],
                                start=(fi == 0), stop=(fi == NF - 1))
                        nc.vector.tensor_scalar(
                            out=qt[:, p, ck * 256:(ck + 1) * 256],
                            in0=psv, scalar1=0.125,
                            scalar2=bq8_sb[:, p:p + 1],
                            op0=Alu.mult, op1=Alu.add)
                return run

            def vproj_thunk(kkt):
                def run():
                    if kkt + 2 < NKT:
                        xv_t[kkt + 2] = xvpool.tile([128, NF, 128], BF16,
                                                    name=f"xv{kkt+2}",
                                                    tag="xv")
                        nc.scalar.dma_start(
                            out=xv_t[kkt + 2][:],
                            in_=xvT_r[:, :, (kkt + 2) * 128:(kkt + 3) * 128])
                    psv = xpspool.tile([128, 512], F32, name="vps",
                                        tag="xps")
                    for fi in range(NF):
                        nc.tensor.matmul(
                            psv, xv_t[kkt][:, fi, :], wv_sb[:, fi, :],
                            start=(fi == 0), stop=(fi == NF - 1))
                    nc.vector.tensor_copy(
                        out=vaug[:, kkt, :, 0:64],
                        in_=psv.rearrange("p (h d) -> p h d", d=64))
                    emitted_v.add(kkt)
                return run

            def nmload_thunk(kkg):
                def run():
                    t = nmpool.tile([128, 2, 1024], BF16, name=f"nm1{kkg}",
                                    tag="nm")
                    nm_t[(1, kkg)] = t
                    for half in range(2):
                        nc.sync.dma_start(
                            out=t[:, half:half + 1, :],
                            in_=notmT_r[:, kkg * 2 + half:
                                        kkg * 2 + half + 1, 1024:2048])
                return run

            def outproj_thunk(qti, et, alt=False):
                qoff = qti * 1024
                def run():
                    for qh in range(2):
                        if alt and qh == 1:
                            pst = ovpool.tile([128, 4, 128], F32,
                                              name="ops2", tag="ov")
                            psv = pst.rearrange("p a b -> p (a b)")
                        elif alt and et % 2 == 1:
                            pst = stpool.tile([128, 1024], F32,
                                              name="ops3", tag="st")
                            psv = pst[:, 0:512]
                        else:
                            psv = xpspool.tile([128, 512], F32, name="ops",
                                               tag="xps")
                        for dc in range(NP):
                            nc.tensor.matmul(
                                psv, wo_sb[:, dc, et * 128:(et + 1) * 128],
                                ott[:, dc,
                                    qoff + qh * 512:qoff + (qh + 1) * 512],
                                start=(dc == 0), stop=(dc == NP - 1))
                        stg = ostg.tile([128, 512], F32, name="cstg")
                        nc.vector.tensor_scalar(
                            out=stg[:], in0=psv,
                            scalar1=bo2_sb[:, et:et + 1], scalar2=None,
                            op0=Alu.add)
                        nc.sync.dma_start(
                            out=outT[et * 128:(et + 1) * 128,
                                     qoff + qh * 512:qoff + (qh + 1) * 512],
                            in_=stg[:])
                return run

            emitted_v = set()

            def vproj_if_needed(kkt):
                if kkt not in emitted_v:
                    vproj_thunk(kkt)()

            # ---------------- upfront PE: minimum to start h0 ------------
            kproj_thunk(0, 0)()
            for ck in range(4):
                qproj_thunk(ck, 0)()

            # extras: {(qti,h): [(slot, thunk), ...]} (V is pulled on demand
            # by the PV drain, so only K/Q/nm/outproj are slotted here)
            X = {
                (0, 0): [(1, kproj_thunk(0, 1)), (5, kproj_thunk(0, 2)),
                         (9, kproj_thunk(0, 3))],
                (0, 1): [(0, kproj_thunk(1, 0)), (4, kproj_thunk(1, 1)),
                         (8, kproj_thunk(1, 2)), (12, kproj_thunk(1, 3))],
                (0, 2): [(2, kproj_thunk(2, 0)), (6, kproj_thunk(2, 1)),
                         (10, kproj_thunk(2, 2)), (14, kproj_thunk(2, 3))],
                (0, 3): [(1, qproj_thunk(0, 1)), (5, qproj_thunk(1, 1)),
                         (9, qproj_thunk(2, 1)), (13, qproj_thunk(3, 1))],
                (0, 4): [(1, kproj_thunk(3, 0)), (5, kproj_thunk(3, 1)),
                         (9, kproj_thunk(3, 2)), (13, kproj_thunk(3, 3))],
                (0, 5): [(1, qproj_thunk(4, 0)), (5, qproj_thunk(4, 1)),
                         (9, qproj_thunk(5, 0)), (13, qproj_thunk(5, 1)),
                         (11, nmload_thunk(0)), (15, nmload_thunk(1))],
                (0, 6): [(1, qproj_thunk(6, 0)), (5, qproj_thunk(6, 1)),
                         (3, nmload_thunk(2)), (7, nmload_thunk(3)),
                         (9, nmload_thunk(4)), (13, nmload_thunk(5))],
                (0, 7): [(1, qproj_thunk(7, 0)), (5, qproj_thunk(7, 1)),
                         (9, nmload_thunk(6)), (13, nmload_thunk(7))],
                (1, 0): [(2, outproj_thunk(0, 0)), (10, outproj_thunk(0, 1))],
                (1, 1): [(2, outproj_thunk(0, 2)), (10, outproj_thunk(0, 3))],
                (1, 2): [(2, outproj_thunk(0, 4)), (10, outproj_thunk(0, 5))],
                (1, 3): [(2, outproj_thunk(0, 6)), (10, outproj_thunk(0, 7))],
            }

            # ---------------- spine with pending-PV queue ----------------
            pvq = []   # entries: [qti, h, kkt, pt, ov, remaining-first-flag]
            seg_left = {}  # (qti,h) -> count of unemitted PVs

            def head_finish(qti, h, ov, otn):
                p, lo = h // 2, (h % 2) * 64
                qoff = qti * 1024
                onorm = onpool.tile([128, 8, 65], F32, name="onorm")
                nc.vector.tensor_copy(out=onorm[:, 0:4, :],
                                      in_=ov["A"][:, :, 0:65])
                nc.vector.tensor_copy(out=onorm[:, 4:8, :],
                                      in_=ov["B"][:, :, 0:65])
                for qs in range(8):
                    nc.gpsimd.normalize_recip(
                        otn[:, qs, lo:lo + 64],
                        onorm[:, qs, 0:64],
                        onorm[:, qs, 64:65])
                if h % 2 == 1:
                    for qs in range(8):
                        nc.sync.dma_start_transpose(
                            ott[:, p,
                                qoff + qs * 128:qoff + (qs + 1) * 128],
                            otn[:, qs, :])

            def emit_one_pv():
                qti, h, kkt, ptt, ov, otn = pvq.pop(0)
                if "A" not in ov:
                    ov["A"] = ovpool.tile([128, 4, 128], F32, name="ovA",
                                          tag="ov")
                    ov["B"] = ovpool.tile([128, 4, 128], F32, name="ovB",
                                          tag="ov")
                for qs in range(8):
                    tgt = ov["A"] if qs < 4 else ov["B"]
                    nc.tensor.matmul(
                        tgt[:, qs % 4, 0:65],
                        ptt[:, qs * 128:(qs + 1) * 128],
                        vaug[:, kkt, h, :],
                        start=(kkt == 0 and qs % 4 == 0),
                        stop=(kkt == NKT - 1 and qs % 4 == 3),
                        skip_group_check=qs % 4 not in (0, 3))
                seg_left[(qti, h)] -= 1
                if seg_left[(qti, h)] == 0:
                    head_finish(qti, h, ov, otn)

            def drain_pv(limit_pending):
                while len(pvq) > limit_pending:
                    vproj_if_needed(pvq[0][2])
                    emit_one_pv()

            otn = None
            for qti in range(2):
                qoff = qti * 1024
                for h in range(8):
                    p, lo = h // 2, (h % 2) * 64
                    slots = [[] for _ in range(16)]
                    for s, th in X.get((qti, h), []):
                        slots[s].append(th)
                    ov = {}
                    if h % 2 == 0:
                        otn = otnpool.tile([128, 8, 128], BF16,
                                           name="otn", tag="otn")
                    seg_left[(qti, h)] = NKT
                    lag = 0 if (qti, h) == (1, 7) else LAG
                    for kkt in range(NKT):
                        drain_pv(lag)
                        with tc.high_priority():
                            st = stpool.tile([128, 1024], F32, name="st",
                                             tag="st")
                            for j in range(2):
                                nc.tensor.matmul(
                                    st[:, j * 512:(j + 1) * 512],
                                    kt[lo:lo + 64, p,
                                       kkt * 128:(kkt + 1) * 128],
                                    qt[lo:lo + 64, p,
                                       qoff + j * 512:qoff + (j + 1) * 512],
                                    start=True, stop=True,
                                    tile_position=(lo, 0))
                            pt = ptpool.tile([128, 1024], BF16, name="pt")
                            nc.scalar.activation(pt[:], st[:], Act.Exp)
                            nmt = nm_t[(qti, kkt // 2)]
                            nc.vector.tensor_tensor(
                                out=pt[:], in0=pt[:],
                                in1=nmt[:, kkt % 2, :], op=Alu.mult)
                        pvq.append([qti, h, kkt, pt, ov, otn])
                        for th in slots[kkt]:
                            th()
                    drain_pv(0)
            # tail: q-tile-1 output projection (extra PSUM depth via the
            # now-idle ov pool keeps the PE fed and at full p-state)
            for et in range(NF):
                outproj_thunk(1, et, alt=True)()
    nc.compile()
    return nc


def _get_nc():
    if "nc" not in _STATE:
        _STATE["nc"] = build_nc()
    return _STATE["nc"]


def kernel(query, key, value, mask, Wq, bq, Wk, bk, Wv, bv, Wo, bo):
    query = np.asarray(query, dtype=np.float32)
    key = np.asarray(key, dtype=np.float32)
    value = np.asarray(value, dtype=np.float32)
    mask = np.asarray(mask)
    Wq = np.asarray(Wq, dtype=np.float32)
    Wk = np.asarray(Wk, dtype=np.float32)
    Wv = np.asarray(Wv, dtype=np.float32)
    Wo = np.asarray(Wo, dtype=np.float32)
    bq = np.asarray(bq, dtype=np.float32)
    bk = np.asarray(bk, dtype=np.float32)
    bv = np.asarray(bv, dtype=np.float32)
    bo = np.asarray(bo, dtype=np.float32)

    bf = ml_dtypes.bfloat16
    xq_b, xk_b, xv_b, nm_b = [], [], [], []
    for b in range(4):
        xq_b.append(np.ascontiguousarray(query[b].T).astype(bf))
        xk_b.append(np.ascontiguousarray(key[b].T).astype(bf))
        xv_b.append(np.ascontiguousarray(value[b].T).astype(bf))
        nm_b.append(np.ascontiguousarray(
            (~mask[b, 0]).T.astype(bf)))
    wq_g, wk_g, wv_g, wo_g, bq_g, bk_g, bo_g = [], [], [], [], [], [], []
    for hg in range(2):
        dsl = slice(hg * 512, (hg + 1) * 512)
        wq_g.append(np.ascontiguousarray(Wq.T[:, dsl]).astype(bf))
        wk_g.append(np.ascontiguousarray(Wk.T[:, dsl]).astype(bf))
        wv_g.append(np.ascontiguousarray(Wv.T[:, dsl]).astype(bf))
        wo_g.append(np.ascontiguousarray(Wo[:, dsl].T).astype(bf))
        bq_g.append(np.ascontiguousarray((bq[dsl] / 8.0).reshape(NP, 128).T))
        bk_g.append(np.ascontiguousarray(bk[dsl].reshape(NP, 128).T))
        bo_g.append(np.ascontiguousarray(
            (bo / 2.0 + Wo[:, dsl] @ bv[dsl]).reshape(NF, 128).T))

    in_maps = []
    for c in range(N_CORES):
        b, hg = c // 2, c % 2
        in_maps.append({
            "xqT": xq_b[b], "xkT": xk_b[b], "xvT": xv_b[b],
            "wqT": wq_g[hg], "wkT": wk_g[hg], "wvT": wv_g[hg],
            "woT": wo_g[hg],
            "bq8": bq_g[hg], "bk_l": bk_g[hg], "bo2": bo_g[hg],
            "notmT": nm_b[b],
        })

    nc = _get_nc()
    res = run_bass_kernel_spmd(nc, in_maps, core_ids=list(range(N_CORES)))
    out = np.empty((4, SQ, EMBED), dtype=np.float32)
    for b in range(4):
        acc = res.results[2 * b]["outT"] + res.results[2 * b + 1]["outT"]
        out[b] = acc.T
    return out
